# revision 46
# baseline (speedup 1.0000x reference)
"""Trainium2 Bass kernel for nn_AttentionPoolDown.

Structure exploited:
  * reference returns out[:, :, :P, :] -- only the P=128 pool queries matter,
    attending over L = P + T = 2176 keys.
  * ALiBi-style bias -slope*|ridx_q - ridx_k| decomposes over integer region
    ids (0..32) as |a-b| = a + b - 2*sum_t 1[a>=t]*1[b>=t], so the entire
    logits tensor scale*QK^T + bias is ONE matmul with an augmented
    contraction dim of 98: [64 roped dims | 32 indicator dims | 1 | ridx].
  * scores are bounded (|logits| < ~40) so softmax needs no max-subtraction:
    p = exp(logits), out = (p @ V) / (p @ 1).  Appending a ones-column to V
    yields the row sums for free in the same PV matmul.
  * Everything is computed in transposed layout-B ([keys, queries] chunks of
    128) so no on-chip transposes are ever needed.  The PV matmul runs with
    p stationary and V moving (65 moving rows per chunk instead of 128) and
    lands the accumulator directly in the output's [q, d] layout.
  * The 34 augmented contraction rows on the key side (indicators/ones/ridx)
    are head-INDEPENDENT: they are DMA'd once per core and replicated to the
    other 3 head windows by the otherwise-idle DVE (4x bf16 copy mode),
    cutting HBM traffic by ~15%.
  * bf16 storage + matmuls (accumulation in fp32 PSUM); rel err ~6e-3.

Sharding: B*H = 32 (b,h) pairs, 4 per core; core c handles b = c//4,
heads 4*(c%4)..4*(c%4)+3.

The walrus build here rejects instructions carrying more than ONE semaphore
wait, and Tile converts any same-engine data dependency into a "wait for all
prior own-engine instructions" self-wait.  The structure below funnels every
instruction's dependencies through a single semaphore: dep-free warmups
absorb preamble-barrier ticks, tiny claimer ops absorb DMA/cross-engine
waits in program order (Tile elides the now-redundant waits on the real
consumers), PSUM tiles are evacuated through DVE only, all big SBUF tensors
are statically placed (no pool-rotation WAR), and the out-DMAs ride gpsimd
SWDGE queues (off the HWDGE semaphore pool).
"""

import os
import numpy as np
import ml_dtypes

B, H, D, T = 2, 16, 64, 2048
MAX_N, R = 32, 4
P = MAX_N * R           # 128 pool tokens (these are the queries)
L = P + T               # 2176 keys
THETA = 10000.0
SCALE = 1.0 / np.sqrt(D)
AUG = 98                # 64 + 32 + 2 augmented contraction
NCHUNK = L // 128       # 17 key chunks
NCORES = 8
PAIRS = (B * H) // NCORES   # 4 (b,h) pairs per core

WKQ = P + L             # 2304 cols per head window in the KQ tile
WVA = NCHUNK * 65       # 1105 cols per head window in the VA tile
WPT = NCHUNK * P        # 2176 cols per head window in the PT tile

_COMPILED = {}

# experiment toggles
SKIP_INIT_BARRIER = bool(int(os.environ.get("K_SKIP_INIT_BARRIER", "0")))


def _rope_pair(x, pos):
    """x: [..., L, 32], pos: [..., L] -> rotary split-half, Dh=32."""
    inv = (1.0 / (THETA ** (np.arange(0, 32, dtype=np.float32)[::2] / 32.0))).astype(np.float32)
    ang = pos[..., :, None] * inv                       # [..., L, 16]
    c, s = np.cos(ang), np.sin(ang)
    x1, x2 = x[..., :16], x[..., 16:]
    return np.concatenate([x1 * c - x2 * s, x1 * s + x2 * c], axis=-1)


def _host_prep(pool_q, pool_k, pool_v, x_q, x_k, x_v, bias_slopes, regions):
    """Returns kqa [B,H,98,WKQ] bf16, va [B,H,128,WVA] bf16."""
    regions = regions.astype(np.int32)
    n_ids = np.arange(1, MAX_N + 1, dtype=np.int32)

    eq = regions[:, None, :] == n_ids[None, :, None]            # [B,32,T]
    starts = np.argmax(eq, axis=-1).astype(np.float32)          # [B,32]
    pool_gpos = (starts[..., None] + 0.5 * np.arange(R, dtype=np.float32)).reshape(B, P)
    gpos = np.concatenate(
        [pool_gpos, np.broadcast_to(np.arange(T, dtype=np.float32), (B, T))], -1)
    pool_ridx = np.broadcast_to(np.repeat(n_ids, R), (B, P))
    ridx = np.concatenate([pool_ridx, regions], -1).astype(np.float32)   # [B,L]

    k = np.concatenate([pool_k, x_k], axis=2)                   # [B,H,L,64]
    gpos_b = gpos[:, None]                                      # [B,1,L]
    ridx_b = ridx[:, None]
    kr = np.concatenate(
        [_rope_pair(k[..., :32], gpos_b), _rope_pair(k[..., 32:], ridx_b)], -1)
    qr = np.concatenate(
        [_rope_pair(pool_q[..., :32], gpos_b[..., :P]),
         _rope_pair(pool_q[..., 32:], ridx_b[..., :P])], -1)    # [B,H,P,64]

    Bind = (ridx[:, None, :] >= n_ids[:, None].astype(np.float32)).astype(np.float32)  # [B,32,L]
    sl = bias_slopes.astype(np.float32)                         # [H]

    kqa = np.empty((B, H, AUG, WKQ), np.float32)
    kqa[:, :, :64, P:] = np.swapaxes(kr, -1, -2)
    kqa[:, :, 64:96, P:] = Bind[:, None]
    kqa[:, :, 96, P:] = 1.0
    kqa[:, :, 97, P:] = ridx[:, None]
    kqa[:, :, :64, :P] = SCALE * np.swapaxes(qr, -1, -2)
    kqa[:, :, 64:96, :P] = 2.0 * sl[None, :, None, None] * Bind[:, None, :, :P]
    kqa[:, :, 96, :P] = -sl[None, :, None] * ridx[:, None, :P]
    kqa[:, :, 97, :P] = -sl[None, :, None]

    v = np.concatenate([pool_v, x_v], axis=2)                   # [B,H,L,64]
    vaug = np.concatenate([v, np.ones((B, H, L, 1), np.float32)], -1)
    va = vaug.reshape(B, H, NCHUNK, 128, 65).transpose(0, 1, 3, 2, 4).reshape(
        B, H, 128, WVA)                                         # [B,H,128,WVA]
    return kqa.astype(ml_dtypes.bfloat16), va.astype(ml_dtypes.bfloat16)


def _patch_tile_drain():
    """The walrus build in this container rejects instructions with more than
    one semaphore wait.  Tile's kernel-tail drain aggregates the whole vector
    clock onto a single Drain -- split those waits across preceding
    single-wait sync-engine nops."""
    import bass_rust
    import concourse.tile as tile
    from concourse.vector_clock import ScopedClock
    if getattr(tile.TileContext, "_drain_split_patched", False):
        return

    def patched(self, tick_clock, wait_clock):
        # The wait-walk nops ride the otherwise-idle Pool engine so they can
        # burn through already-satisfied sems while SP is still occupied
        # issuing the tail out-DMAs; the closing all-engine barrier makes the
        # drain sound even though the drain itself keeps only one wait.
        nc = self.nc
        nops = [nc.sync.nop(nofuse=True) for _ in range(17)]
        drain_inst = nc.sync.drain()
        wait_clock.add_sem_waits(
            drain_inst.ins, ScopedClock({None: tick_clock.global_clock}))
        si = drain_inst.ins.sync_info
        waits = list(si.on_wait) if si is not None else []
        # order the walk by expected fire time (late sems last) so one
        # late-firing semaphore doesn't serialize the remaining waits
        prio = getattr(nc, "_drain_wait_prio", {})
        waits.sort(key=lambda w: prio.get(
            (w.ant_name or "").rsplit("_", 1)[0], 50))
        if len(waits) > 1:
            upd = list(si.on_update)
            assert len(waits) - 1 <= len(nops)
            for nop, w in zip(nops, waits[:-1]):
                old = nop.ins.sync_info
                nupd = list(old.on_update) if old is not None else []
                nop.ins.sync_info = bass_rust.SyncInfo(
                    on_wait=[w], on_update=nupd)
            drain_inst.ins.sync_info = bass_rust.SyncInfo(
                on_wait=[waits[-1]], on_update=upd)
        nc.all_engine_barrier()
        assert self.sems is not None
        popped = nc._tile_sem_poison_stack.pop()
        assert popped is self._sem_poison
        nc.clear_and_free_semaphores(list(self.sems.allocated().values()))
        nc.all_engine_barrier()

    tile.TileContext._drain_and_barrier = patched
    tile.TileContext._drain_split_patched = True


def _patch_skip_init_barrier():
    """Bass.__init__ ends with an all-engine barrier guarding the const-AP
    memsets.  This kernel never reads the const pool (every activation bias
    is an explicit AP), and the ~1us barrier sits directly on the
    first-input-DMA critical path -- skip just that one barrier."""
    import concourse.bass as bass
    if getattr(bass.Bass, "_init_barrier_skip_patched", False):
        return
    orig = bass.Bass.all_engine_barrier

    def patched(self, *, sem_only: bool = False):
        if not getattr(self, "_init_barrier_skipped", False):
            self._init_barrier_skipped = True
            return
        return orig(self, sem_only=sem_only)

    bass.Bass.all_engine_barrier = patched
    bass.Bass._init_barrier_skip_patched = True


def _repair_multi_waits(nc):
    """Walrus rejects instructions with more than one semaphore wait.  Tile's
    scheduler occasionally lands 2-3 waits on one instruction (its internal
    ordering displaces the hand-written claimer ops).  Repair post-hoc: move
    all but one wait of each offender onto the nearest PRECEDING wait-free
    instructions of the same engine.  Moving a wait earlier on the same
    engine only strengthens ordering; cycle-freedom is re-checked by the
    timeline simulation after the build."""
    import bass_rust

    DONOR_TYPES = (
        "InstMatmult", "InstNoOp", "InstTensorCopy", "InstActivation",
        "InstMemset", "InstTensorTensor", "InstTensorScalarPtr",
    )
    f = nc.m.functions[0]
    moved = []
    for blk in f.blocks:
        insts = list(blk.instructions)
        donors = {}                      # engine -> wait-free donor stack
        for i in insts:
            eng = i.engine.name
            si = getattr(i, "sync_info", None)
            ws = list(si.on_wait) if si is not None else []
            if len(ws) <= 1:
                if (not ws) and type(i).__name__ in DONOR_TYPES:
                    donors.setdefault(eng, []).append(i)
                continue
            dl = donors.setdefault(eng, [])

            def is_dma(w):
                return (w.ant_name or "").startswith(("DMAHW", "DMASW"))

            def is_self(w):
                return (w.ant_name or "").startswith(eng + "_")

            # keep preference: self-wait first (never safe to move), then
            # any engine wait; DMA-lane waits are always safe to move
            ws.sort(key=lambda w: (0 if is_dma(w) else (2 if is_self(w) else 1)))
            keep = [ws.pop()]
            while ws and dl:
                d = dl.pop()             # nearest preceding donor first
                dsi = d.sync_info
                dupd = list(dsi.on_update) if dsi is not None else []
                w = ws.pop()
                d.sync_info = bass_rust.SyncInfo(on_wait=[w], on_update=dupd)
                moved.append((i.name, d.name, w.ant_name, w.wait_value,
                              "dma" if is_dma(w) else "eng"))
            assert not ws, (
                f"_repair_multi_waits: no donor for {i.name} ({eng}); "
                f"residual waits {[str(w) for w in ws]}")
            upd = list(si.on_update)
            i.sync_info = bass_rust.SyncInfo(on_wait=keep, on_update=upd)
    return moved


def _build_module():
    import concourse.bass as bass
    import concourse.tile as tile
    from concourse import mybir

    _patch_tile_drain()
    if SKIP_INIT_BARRIER:
        _patch_skip_init_barrier()

    f32 = mybir.dt.float32
    bf16 = mybir.dt.bfloat16
    nc = bass.Bass(num_swdge_queues=2)

    # ---- DRAM parameters (per core) -------------------------------------
    # pair 0's kq window arrives in two pieces so the first score matmuls
    # start before the whole tensor lands; pairs 1-3 rows 0..63 are per-head
    # unique, rows 64..97 of the key part are replicated on-chip.
    kq0a_d = nc.declare_dram_parameter("kq0a", [AUG, P + 8 * 128], bf16, isOutput=False)
    kq0b_d = nc.declare_dram_parameter("kq0b", [AUG, 9 * 128], bf16, isOutput=False)
    kq1_d = nc.declare_dram_parameter("kq1", [64, WKQ], bf16, isOutput=False)
    kq23_d = nc.declare_dram_parameter("kq23", [64, 2 * WKQ], bf16, isOutput=False)
    qaugr_d = nc.declare_dram_parameter("qaugr", [34, 3, P], bf16, isOutput=False)
    va0_d = nc.declare_dram_parameter("va0", [128, WVA], bf16, isOutput=False)
    va1_d = nc.declare_dram_parameter("va1", [128, WVA], bf16, isOutput=False)
    va23_d = nc.declare_dram_parameter("va23", [128, 2 * WVA], bf16, isOutput=False)
    out_d = nc.declare_dram_parameter("outt", [PAIRS, P, 65], f32, isOutput=True)

    GROUPS = [(0, 8), (8, 8), (16, 1)]
    SPLIT0 = P + 8 * 128        # pair-0 kq piece boundary

    # expected semaphore fire order for the drain walk: HWDGE lanes 0..7 are
    # the input DMAs in issue order (all fire mid-kernel); the SWDGE lanes
    # carry the out-DMAs, of which the last two fire after the compute tail.
    nc._drain_wait_prio = {
        "DMAHW0": 0, "DMAHW1": 1, "DMAHW2": 2, "DMAHW3": 3,
        "DMAHW4": 4, "DMAHW5": 5, "DMAHW6": 6, "DMAHW7": 7,
        "Pool": 20, "Activation": 21, "PE": 22, "DVE": 23,
        "DMASW0": 30, "DMASW1": 31, "DMASW2": 32, "DMASW3": 33,
        "DMASW4": 34, "DMASW5": 35, "DMASW6": 36, "DMASW7": 37,
    }

    with tile.TileContext(nc) as tc:
        with (
            tc.tile_pool(name="kq", bufs=1) as kqpool,
            tc.tile_pool(name="va", bufs=1) as vapool,
            tc.tile_pool(name="pt", bufs=1) as ptpool,
            tc.tile_pool(name="os", bufs=1) as ospool,
            tc.tile_pool(name="epool", bufs=4) as epool,
            tc.tile_pool(name="sp8sum", bufs=2, space="PSUM") as sp8sum,
            tc.tile_pool(name="sp1sum", bufs=1, space="PSUM") as sp1sum,
            tc.tile_pool(name="apsum", bufs=2, space="PSUM") as apsum,
            tc.tile_pool(name="dpsum", bufs=1, space="PSUM") as dpsum,
        ):
            # ---- static SBUF tensors ------------------------------------
            KQ = kqpool.tile([AUG, PAIRS * WKQ], bf16, name="KQ")
            VA = vapool.tile([128, PAIRS * WVA], bf16, name="VA")
            PT = ptpool.tile([128, PAIRS * WPT], bf16, name="PT")
            OS = ospool.tile([128, PAIRS * 65], f32, name="OS")

            # ---- dep-free warmups ---------------------------------------
            # dummy-claim matmuls each write their OWN psum byte: a shared
            # target would WAW-chain the claims, letting one late-firing
            # claim displace later ones behind their real consumers in
            # Tile's readiness-ordered schedule.
            dumm_all = dpsum.tile([1, 64], f32)
            dumm_ctr = [0]

            def dumm_slot():
                i = dumm_ctr[0]
                dumm_ctr[0] += 1
                return dumm_all[0:1, i:i + 1]

            dumm = dumm_slot()
            # dummy operand: the framework's const pool is initialized in the
            # preamble (before the entry barrier), so reads are dep-free AND
            # defined -- CoreSim rejects reads of uninitialized SBUF
            CB1 = nc.const_aps.aps[(bf16, 1.0)][0:1, 0:1]
            dwarm0 = epool.tile([1, 1], bf16, tag="dwarm")
            nc.tensor.matmul(dumm, lhsT=CB1, rhs=CB1,
                             start=True, stop=True, skip_group_check=True)
            nc.scalar.copy(dwarm0, CB1)
            # DVE warmup doubles as the exp-bias zero column
            ZB = epool.tile([128, 1], bf16, tag="zb")
            nc.vector.memset(ZB, 0.0)

            # ---- input DMAs (SP -> HWDGE), urgency order ----------------
            # exactly 8: one per HWDGE semaphore lane (a 9th would carry a
            # lane-FIFO wait on top of its data wait -> illegal on walrus)
            nc.sync.dma_start(out=KQ[:, :SPLIT0], in_=kq0a_d[:])
            nc.sync.dma_start(out=KQ[:, SPLIT0:WKQ], in_=kq0b_d[:])
            nc.sync.dma_start(out=KQ[:64, WKQ:2 * WKQ], in_=kq1_d[:])
            qaug_dst = KQ[64:, WKQ:].rearrange("p (w c) -> p w c", c=WKQ)[:, :, :P]
            nc.sync.dma_start(out=qaug_dst, in_=qaugr_d[:])
            nc.sync.dma_start(out=VA[:, :WVA], in_=va0_d[:])
            nc.sync.dma_start(out=KQ[:64, 2 * WKQ:], in_=kq23_d[:])
            nc.sync.dma_start(out=VA[:, WVA:2 * WVA], in_=va1_d[:])
            nc.sync.dma_start(out=VA[:, 2 * WVA:], in_=va23_d[:])

            # ---- ACT claims: zero bias column + exp-table preload -------
            dume = epool.tile([1, 1], f32, tag="dume")
            nc.scalar.copy(dume, ZB[0:1, :])
            dume2 = epool.tile([1, 1], f32, tag="dume2")
            nc.scalar.activation(dume2, ZB[0:1, :],
                                 mybir.ActivationFunctionType.Exp,
                                 bias=ZB[0:1, :])

            # ---- DVE: claim pair-0 kq pieces, then replicate aug rows ---
            dumv = epool.tile([1, 1], bf16, tag="dumv")
            nc.vector.tensor_copy(dumv, KQ[64:65, P:P + 1])              # kq0a
            nc.vector.tensor_copy(dumv, KQ[64:65, SPLIT0:SPLIT0 + 1])    # kq0b
            nc.vector.nop(nofuse=True)
            for j in range(1, PAIRS):
                nc.vector.nop(nofuse=True)
                nc.vector.tensor_copy(
                    KQ[64:, j * WKQ + P:(j + 1) * WKQ],
                    KQ[64:, P:WKQ])

            GL = [GROUPS] * PAIRS
            state = {}

            def claim(engine_matmul_src):
                nc.tensor.matmul(dumm_slot(), lhsT=engine_matmul_src,
                                 rhs=engine_matmul_src,
                                 start=True, stop=True, skip_group_check=True)

            def scores_group(j, gi):
                st = state.setdefault(j, {})
                c0, ng = GL[j][gi]
                w = j * WKQ
                qhat = KQ[:, w:w + P]
                if j == 0:
                    # pair 0 lands in two pieces; claim each as it is used
                    if gi == 0:
                        claim(KQ[0:1, 0:1])
                    elif gi == 1:
                        claim(KQ[0:1, SPLIT0:SPLIT0 + 1])
                elif gi == 0:
                    claim(KQ[0:1, w:w + 1])                # per-head kq DMA
                    if j == 1:
                        claim(KQ[64:65, w:w + 1])          # qaugr DMA
                    claim(KQ[64:65, w + P:w + P + 1])      # DVE replica j
                # dep-free donor nops: _repair_multi_waits parks displaced
                # waits here, adjacent to the instructions that need them
                nc.tensor.nop(nofuse=True)
                nc.tensor.nop(nofuse=True)
                sp = (sp8sum.tile([128, 1024], f32, name="sp8") if ng > 1
                      else sp1sum.tile([128, 128], f32, name="sp1"))
                for i in range(ng):
                    c = c0 + i
                    nc.tensor.matmul(
                        sp[:, i * P:(i + 1) * P],
                        lhsT=KQ[:, w + P + c * 128:w + P + (c + 1) * 128],
                        rhs=qhat,
                        start=True, stop=True)
                st.setdefault("sps", []).append(sp)

            def exp_group(j, gi):
                st = state[j]
                c0, ng = GL[j][gi]
                sp = st["sps"][gi]
                nc.scalar.activation(
                    PT[:, j * WPT + c0 * P:j * WPT + (c0 + ng) * P],
                    sp[:, :ng * P],
                    mybir.ActivationFunctionType.Exp, bias=ZB)

            def pv_group(j, gi):
                st = state[j]
                c0, ng = GL[j][gi]
                if gi == 0:
                    # claims: va DMA (lane changes at pairs 0,1,2), acc WAR
                    if j == 0:
                        claim(VA[0:1, 0:1])
                    elif j == 1:
                        claim(VA[0:1, WVA:WVA + 1])
                    elif j == 2:
                        claim(VA[0:1, 2 * WVA:2 * WVA + 1])
                    st["acc"] = apsum.tile([P, 65], f32, name="acc")
                    nc.tensor.nop(nofuse=True)
                    if j >= 2:
                        # absorb the WAR wait on the recycled acc buffer
                        nc.tensor.matmul(st["acc"][0:1, 64:65],
                                         lhsT=CB1, rhs=CB1,
                                         start=True, stop=True,
                                         skip_group_check=True)
                for i in range(ng):
                    c = c0 + i
                    nc.tensor.matmul(
                        st["acc"],
                        lhsT=PT[:, j * WPT + c * P:j * WPT + (c + 1) * P],
                        rhs=VA[:, j * WVA + c * 65:j * WVA + (c + 1) * 65],
                        start=(c == 0), stop=(c == NCHUNK - 1))

            def evac_pair(j):
                st = state[j]
                acc_sb = OS[:, j * 65:(j + 1) * 65]
                nc.vector.tensor_copy(acc_sb, st["acc"])
                nc.tensor.matmul(dumm if j == PAIRS - 1 else dumm_slot(),
                                 lhsT=CB1, rhs=CB1,
                                 start=True, stop=True,
                                 skip_group_check=True)
                # gpsimd claimer absorbs the DVE data wait so the SWDGE
                # dma_start itself carries no semaphore wait
                dumg = epool.tile([1, 1], f32, tag=f"dumg{j}", name=f"dumg{j}")
                nc.gpsimd.tensor_copy(out=dumg, in_=acc_sb[0:1, 0:1])
                nc.gpsimd.dma_start(out=out_d[j], in_=acc_sb)

            # software pipeline: scores of pair j+1 interleave with exp/PV of j
            for gi in range(len(GL[0])):
                scores_group(0, gi)
            for j in range(PAIRS):
                nxt = GL[j + 1] if j + 1 < PAIRS else []
                for gi in range(len(GL[j])):
                    exp_group(j, gi)
                    pv_group(j, gi)
                    if gi < len(nxt):
                        scores_group(j + 1, gi)
                evac_pair(j)

    _repair_multi_waits(nc)
    return nc


def _get_nc():
    if "nc" not in _COMPILED:
        _COMPILED["nc"] = _build_module()
    return _COMPILED["nc"]


def kernel(pool_q, pool_k, pool_v, x_q, x_k, x_v, bias_slopes, regions,
           t_mask, n_mask, max_n):
    from concourse.bass_utils import run_bass_kernel_spmd

    kqa, va = _host_prep(
        np.asarray(pool_q, np.float32), np.asarray(pool_k, np.float32),
        np.asarray(pool_v, np.float32), np.asarray(x_q, np.float32),
        np.asarray(x_k, np.float32), np.asarray(x_v, np.float32),
        np.asarray(bias_slopes, np.float32), np.asarray(regions))

    SPLIT0 = P + 8 * 128
    in_maps = []
    for c in range(NCORES):
        b, h0 = c // 4, 4 * (c % 4)
        kq = kqa[b]                                     # [H, 98, WKQ]
        in_maps.append({
            "kq0a": np.ascontiguousarray(kq[h0, :, :SPLIT0]),
            "kq0b": np.ascontiguousarray(kq[h0, :, SPLIT0:]),
            "kq1": np.ascontiguousarray(kq[h0 + 1, :64]),
            "kq23": np.ascontiguousarray(
                np.swapaxes(kq[h0 + 2:h0 + 4, :64], 0, 1).reshape(64, 2 * WKQ)),
            "qaugr": np.ascontiguousarray(
                np.swapaxes(kq[h0 + 1:h0 + 4, 64:, :P], 0, 1)),
            "va0": np.ascontiguousarray(va[b, h0]),
            "va1": np.ascontiguousarray(va[b, h0 + 1]),
            "va23": np.ascontiguousarray(
                np.swapaxes(va[b, h0 + 2:h0 + 4], 0, 1).reshape(128, 2 * WVA)),
        })

    nc = _get_nc()
    res = run_bass_kernel_spmd(
        nc, in_maps, core_ids=list(range(NCORES)),
        trace=bool(int(os.environ.get("KERNEL_TRACE", "0"))))
    _COMPILED["last_result"] = res

    out = np.empty((B, H, P, 64), np.float32)
    for c in range(NCORES):
        b, h0 = c // 4, 4 * (c % 4)
        ot = res.results[c]["outt"]                        # [PAIRS, P, 65]
        out[b, h0:h0 + PAIRS] = ot[:, :, :64] / ot[:, :, 64:65]
    return out


# revision 49
# speedup vs baseline: 1.0660x; 1.0660x over previous
"""Trainium2 Bass kernel for nn_AttentionPoolDown.

Structure exploited:
  * reference returns out[:, :, :P, :] -- only the P=128 pool queries matter,
    attending over L = P + T = 2176 keys.
  * ALiBi-style bias -slope*|ridx_q - ridx_k| decomposes over integer region
    ids (0..32) as |a-b| = a + b - 2*sum_t 1[a>=t]*1[b>=t], so the entire
    logits tensor scale*QK^T + bias is ONE matmul with an augmented
    contraction dim of 98: [64 roped dims | 32 indicator dims | 1 | ridx].
  * scores are bounded (|logits| < ~40) so softmax needs no max-subtraction:
    p = exp(logits), out = (p @ V) / (p @ 1).  Appending a ones-column to V
    yields the row sums for free in the same PV matmul.
  * Everything is computed in transposed layout-B ([keys, queries] chunks of
    128) so no on-chip transposes are ever needed.  The PV matmul runs with
    p stationary and V moving (65 moving rows per chunk instead of 128) and
    lands the accumulator directly in the output's [q, d] layout.
  * The 34 augmented contraction rows on the key side (indicators/ones/ridx)
    are head-INDEPENDENT: they are DMA'd once per core and replicated to the
    other 3 head windows by the otherwise-idle DVE (4x bf16 copy mode),
    cutting HBM traffic by ~15%.
  * bf16 storage + matmuls (accumulation in fp32 PSUM); rel err ~6e-3.

Sharding: B*H = 32 (b,h) pairs, 4 per core; core c handles b = c//4,
heads 4*(c%4)..4*(c%4)+3.

The walrus build here rejects instructions carrying more than ONE semaphore
wait, and Tile converts any same-engine data dependency into a "wait for all
prior own-engine instructions" self-wait.  The structure below funnels every
instruction's dependencies through a single semaphore: dep-free warmups
absorb preamble-barrier ticks, tiny claimer ops absorb DMA/cross-engine
waits in program order (Tile elides the now-redundant waits on the real
consumers), PSUM tiles are evacuated through DVE only, all big SBUF tensors
are statically placed (no pool-rotation WAR), and the out-DMAs ride gpsimd
SWDGE queues (off the HWDGE semaphore pool).
"""

import os
import numpy as np
import ml_dtypes

B, H, D, T = 2, 16, 64, 2048
MAX_N, R = 32, 4
P = MAX_N * R           # 128 pool tokens (these are the queries)
L = P + T               # 2176 keys
THETA = 10000.0
SCALE = 1.0 / np.sqrt(D)
AUG = 98                # 64 + 32 + 2 augmented contraction
NCHUNK = L // 128       # 17 key chunks
NCORES = 8
PAIRS = (B * H) // NCORES   # 4 (b,h) pairs per core

WKQ = P + L             # 2304 cols per head window in the KQ tile
WVA = NCHUNK * 65       # 1105 cols per head window in the VA tile
WPT = NCHUNK * P        # 2176 cols per head window in the PT tile

_COMPILED = {}

# experiment toggles
SKIP_INIT_BARRIER = bool(int(os.environ.get("K_SKIP_INIT_BARRIER", "0")))
DEDUP = bool(int(os.environ.get("K_DEDUP", "1")))


def _rope_pair(x, pos):
    """x: [..., L, 32], pos: [..., L] -> rotary split-half, Dh=32."""
    inv = (1.0 / (THETA ** (np.arange(0, 32, dtype=np.float32)[::2] / 32.0))).astype(np.float32)
    ang = pos[..., :, None] * inv                       # [..., L, 16]
    c, s = np.cos(ang), np.sin(ang)
    x1, x2 = x[..., :16], x[..., 16:]
    return np.concatenate([x1 * c - x2 * s, x1 * s + x2 * c], axis=-1)


def _host_prep(pool_q, pool_k, pool_v, x_q, x_k, x_v, bias_slopes, regions):
    """Returns kqa [B,H,98,WKQ] bf16, va [B,H,128,WVA] bf16."""
    regions = regions.astype(np.int32)
    n_ids = np.arange(1, MAX_N + 1, dtype=np.int32)

    eq = regions[:, None, :] == n_ids[None, :, None]            # [B,32,T]
    starts = np.argmax(eq, axis=-1).astype(np.float32)          # [B,32]
    pool_gpos = (starts[..., None] + 0.5 * np.arange(R, dtype=np.float32)).reshape(B, P)
    gpos = np.concatenate(
        [pool_gpos, np.broadcast_to(np.arange(T, dtype=np.float32), (B, T))], -1)
    pool_ridx = np.broadcast_to(np.repeat(n_ids, R), (B, P))
    ridx = np.concatenate([pool_ridx, regions], -1).astype(np.float32)   # [B,L]

    k = np.concatenate([pool_k, x_k], axis=2)                   # [B,H,L,64]
    gpos_b = gpos[:, None]                                      # [B,1,L]
    ridx_b = ridx[:, None]
    kr = np.concatenate(
        [_rope_pair(k[..., :32], gpos_b), _rope_pair(k[..., 32:], ridx_b)], -1)
    qr = np.concatenate(
        [_rope_pair(pool_q[..., :32], gpos_b[..., :P]),
         _rope_pair(pool_q[..., 32:], ridx_b[..., :P])], -1)    # [B,H,P,64]

    Bind = (ridx[:, None, :] >= n_ids[:, None].astype(np.float32)).astype(np.float32)  # [B,32,L]
    sl = bias_slopes.astype(np.float32)                         # [H]

    kqa = np.empty((B, H, AUG, WKQ), np.float32)
    kqa[:, :, :64, P:] = np.swapaxes(kr, -1, -2)
    kqa[:, :, 64:96, P:] = Bind[:, None]
    kqa[:, :, 96, P:] = 1.0
    kqa[:, :, 97, P:] = ridx[:, None]
    kqa[:, :, :64, :P] = SCALE * np.swapaxes(qr, -1, -2)
    kqa[:, :, 64:96, :P] = 2.0 * sl[None, :, None, None] * Bind[:, None, :, :P]
    kqa[:, :, 96, :P] = -sl[None, :, None] * ridx[:, None, :P]
    kqa[:, :, 97, :P] = -sl[None, :, None]

    v = np.concatenate([pool_v, x_v], axis=2)                   # [B,H,L,64]
    vaug = np.concatenate([v, np.ones((B, H, L, 1), np.float32)], -1)
    va = vaug.reshape(B, H, NCHUNK, 128, 65).transpose(0, 1, 3, 2, 4).reshape(
        B, H, 128, WVA)                                         # [B,H,128,WVA]
    return kqa.astype(ml_dtypes.bfloat16), va.astype(ml_dtypes.bfloat16)


def _patch_tile_drain():
    """The walrus build in this container rejects instructions with more than
    one semaphore wait.  Tile's kernel-tail drain aggregates the whole vector
    clock onto a single Drain -- split those waits across preceding
    single-wait sync-engine nops."""
    import bass_rust
    import concourse.tile as tile
    from concourse.vector_clock import ScopedClock
    if getattr(tile.TileContext, "_drain_split_patched", False):
        return

    def patched(self, tick_clock, wait_clock):
        # The wait-walk nops ride the otherwise-idle Pool engine so they can
        # burn through already-satisfied sems while SP is still occupied
        # issuing the tail out-DMAs; the closing all-engine barrier makes the
        # drain sound even though the drain itself keeps only one wait.
        nc = self.nc
        nops = [nc.sync.nop(nofuse=True) for _ in range(17)]
        drain_inst = nc.sync.drain()
        wait_clock.add_sem_waits(
            drain_inst.ins, ScopedClock({None: tick_clock.global_clock}))
        si = drain_inst.ins.sync_info
        waits = list(si.on_wait) if si is not None else []
        # order the walk by expected fire time (late sems last) so one
        # late-firing semaphore doesn't serialize the remaining waits
        prio = getattr(nc, "_drain_wait_prio", {})
        waits.sort(key=lambda w: prio.get(
            (w.ant_name or "").rsplit("_", 1)[0], 50))
        if len(waits) > 1:
            upd = list(si.on_update)
            assert len(waits) - 1 <= len(nops)
            for nop, w in zip(nops, waits[:-1]):
                old = nop.ins.sync_info
                nupd = list(old.on_update) if old is not None else []
                nop.ins.sync_info = bass_rust.SyncInfo(
                    on_wait=[w], on_update=nupd)
            drain_inst.ins.sync_info = bass_rust.SyncInfo(
                on_wait=[waits[-1]], on_update=upd)
        nc.all_engine_barrier()
        assert self.sems is not None
        popped = nc._tile_sem_poison_stack.pop()
        assert popped is self._sem_poison
        nc.clear_and_free_semaphores(list(self.sems.allocated().values()))
        nc.all_engine_barrier()

    tile.TileContext._drain_and_barrier = patched
    tile.TileContext._drain_split_patched = True


def _patch_skip_init_barrier():
    """Bass.__init__ ends with an all-engine barrier guarding the const-AP
    memsets.  This kernel never reads the const pool (every activation bias
    is an explicit AP), and the ~1us barrier sits directly on the
    first-input-DMA critical path -- skip just that one barrier."""
    import concourse.bass as bass
    if getattr(bass.Bass, "_init_barrier_skip_patched", False):
        return
    orig = bass.Bass.all_engine_barrier

    def patched(self, *, sem_only: bool = False):
        if not getattr(self, "_init_barrier_skipped", False):
            self._init_barrier_skipped = True
            return
        return orig(self, sem_only=sem_only)

    bass.Bass.all_engine_barrier = patched
    bass.Bass._init_barrier_skip_patched = True


def _repair_multi_waits(nc):
    """Walrus rejects instructions with more than one semaphore wait.  Tile's
    scheduler occasionally lands 2-3 waits on one instruction (its internal
    ordering displaces the hand-written claimer ops).  Repair post-hoc: move
    all but one wait of each offender onto the nearest PRECEDING wait-free
    instructions of the same engine.  Moving a wait earlier on the same
    engine only strengthens ordering; cycle-freedom is re-checked by the
    timeline simulation after the build."""
    import bass_rust

    DONOR_TYPES = (
        "InstMatmult", "InstNoOp", "InstTensorCopy", "InstActivation",
        "InstMemset", "InstTensorTensor", "InstTensorScalarPtr",
    )
    f = nc.m.functions[0]
    moved = []
    for blk in f.blocks:
        insts = list(blk.instructions)
        donors = {}                      # engine -> wait-free donor stack
        for i in insts:
            eng = i.engine.name
            si = getattr(i, "sync_info", None)
            ws = list(si.on_wait) if si is not None else []
            if len(ws) <= 1:
                if (not ws) and type(i).__name__ in DONOR_TYPES:
                    donors.setdefault(eng, []).append(i)
                continue
            dl = donors.setdefault(eng, [])

            def is_dma(w):
                return (w.ant_name or "").startswith(("DMAHW", "DMASW"))

            def is_self(w):
                return (w.ant_name or "").startswith(eng + "_")

            # keep preference: self-wait first (never safe to move), then
            # any engine wait; DMA-lane waits are always safe to move
            ws.sort(key=lambda w: (0 if is_dma(w) else (2 if is_self(w) else 1)))
            keep = [ws.pop()]
            while ws and dl:
                d = dl.pop()             # nearest preceding donor first
                dsi = d.sync_info
                dupd = list(dsi.on_update) if dsi is not None else []
                w = ws.pop()
                d.sync_info = bass_rust.SyncInfo(on_wait=[w], on_update=dupd)
                moved.append((i.name, d.name, w.ant_name, w.wait_value,
                              "dma" if is_dma(w) else "eng"))
            assert not ws, (
                f"_repair_multi_waits: no donor for {i.name} ({eng}); "
                f"residual waits {[str(w) for w in ws]}")
            upd = list(si.on_update)
            i.sync_info = bass_rust.SyncInfo(on_wait=keep, on_update=upd)
    return moved


def _build_module():
    import concourse.bass as bass
    import concourse.tile as tile
    from concourse import mybir

    _patch_tile_drain()
    if SKIP_INIT_BARRIER:
        _patch_skip_init_barrier()

    f32 = mybir.dt.float32
    bf16 = mybir.dt.bfloat16
    nc = bass.Bass(num_swdge_queues=2)

    # ---- DRAM parameters (per core) -------------------------------------
    # pair 0's kq window arrives in two pieces so the first score matmuls
    # start before the whole tensor lands; pairs 1-3 rows 0..63 are per-head
    # unique, rows 64..97 of the key part are replicated on-chip.
    kq0a_d = nc.declare_dram_parameter("kq0a", [AUG, P + 8 * 128], bf16, isOutput=False)
    kq0b_d = nc.declare_dram_parameter("kq0b", [AUG, 9 * 128], bf16, isOutput=False)
    KROWS = 64 if DEDUP else AUG
    kq1_d = nc.declare_dram_parameter("kq1", [KROWS, WKQ], bf16, isOutput=False)
    kq23_d = nc.declare_dram_parameter("kq23", [KROWS, 2 * WKQ], bf16, isOutput=False)
    qaugr_d = (nc.declare_dram_parameter("qaugr", [34, 3, P], bf16, isOutput=False)
               if DEDUP else None)
    va0_d = nc.declare_dram_parameter("va0", [128, WVA], bf16, isOutput=False)
    va1_d = nc.declare_dram_parameter("va1", [128, WVA], bf16, isOutput=False)
    va23_d = nc.declare_dram_parameter("va23", [128, 2 * WVA], bf16, isOutput=False)
    out_d = nc.declare_dram_parameter("outt", [PAIRS, P, 65], f32, isOutput=True)

    GROUPS = [(0, 8), (8, 8), (16, 1)]
    SPLIT0 = P + 8 * 128        # pair-0 kq piece boundary

    # expected semaphore fire order for the drain walk: HWDGE lanes 0..7 are
    # the input DMAs in issue order (all fire mid-kernel); the SWDGE lanes
    # carry the out-DMAs, of which the last two fire after the compute tail.
    nc._drain_wait_prio = {
        "DMAHW0": 0, "DMAHW1": 1, "DMAHW2": 2, "DMAHW3": 3,
        "DMAHW4": 4, "DMAHW5": 5, "DMAHW6": 6, "DMAHW7": 7,
        "Pool": 20, "Activation": 21, "PE": 22, "DVE": 23,
        "DMASW0": 30, "DMASW1": 31, "DMASW2": 32, "DMASW3": 33,
        "DMASW4": 34, "DMASW5": 35, "DMASW6": 36, "DMASW7": 37,
    }

    with tile.TileContext(nc) as tc:
        with (
            tc.tile_pool(name="kq", bufs=1) as kqpool,
            tc.tile_pool(name="va", bufs=1) as vapool,
            tc.tile_pool(name="pt", bufs=1) as ptpool,
            tc.tile_pool(name="os", bufs=1) as ospool,
            tc.tile_pool(name="epool", bufs=4) as epool,
            tc.tile_pool(name="sp8sum", bufs=2, space="PSUM") as sp8sum,
            tc.tile_pool(name="sp1sum", bufs=1, space="PSUM") as sp1sum,
            tc.tile_pool(name="apsum", bufs=2, space="PSUM") as apsum,
            tc.tile_pool(name="dpsum", bufs=1, space="PSUM") as dpsum,
        ):
            # ---- static SBUF tensors ------------------------------------
            KQ = kqpool.tile([AUG, PAIRS * WKQ], bf16, name="KQ")
            VA = vapool.tile([128, PAIRS * WVA], bf16, name="VA")
            PT = ptpool.tile([128, PAIRS * WPT], bf16, name="PT")
            OS = ospool.tile([128, PAIRS * 65], f32, name="OS")

            # ---- dep-free warmups ---------------------------------------
            # dummy-claim matmuls each write their OWN psum byte: a shared
            # target would WAW-chain the claims, letting one late-firing
            # claim displace later ones behind their real consumers in
            # Tile's readiness-ordered schedule.
            dumm_all = dpsum.tile([1, 64], f32)
            dumm_ctr = [0]

            def dumm_slot():
                i = dumm_ctr[0]
                dumm_ctr[0] += 1
                return dumm_all[0:1, i:i + 1]

            dumm = dumm_slot()
            # dummy operand: the framework's const pool is initialized in the
            # preamble (before the entry barrier), so reads are dep-free AND
            # defined -- CoreSim rejects reads of uninitialized SBUF
            CB1 = nc.const_aps.aps[(bf16, 1.0)][0:1, 0:1]
            dwarm0 = epool.tile([1, 1], bf16, tag="dwarm")
            nc.tensor.matmul(dumm, lhsT=CB1, rhs=CB1,
                             start=True, stop=True, skip_group_check=True)
            nc.scalar.copy(dwarm0, CB1)
            # DVE warmup doubles as the exp-bias zero column
            ZB = epool.tile([128, 1], bf16, tag="zb")
            nc.vector.memset(ZB, 0.0)

            # ---- input DMAs (SP -> HWDGE), urgency order ----------------
            # exactly 8: one per HWDGE semaphore lane (a 9th would carry a
            # lane-FIFO wait on top of its data wait -> illegal on walrus)
            nc.sync.dma_start(out=KQ[:, :SPLIT0], in_=kq0a_d[:])
            nc.sync.dma_start(out=KQ[:, SPLIT0:WKQ], in_=kq0b_d[:])
            nc.sync.dma_start(out=KQ[:KROWS, WKQ:2 * WKQ], in_=kq1_d[:])
            if DEDUP:
                qaug_dst = KQ[64:, WKQ:].rearrange(
                    "p (w c) -> p w c", c=WKQ)[:, :, :P]
                nc.sync.dma_start(out=qaug_dst, in_=qaugr_d[:])
            nc.sync.dma_start(out=VA[:, :WVA], in_=va0_d[:])
            nc.sync.dma_start(out=KQ[:KROWS, 2 * WKQ:], in_=kq23_d[:])
            nc.sync.dma_start(out=VA[:, WVA:2 * WVA], in_=va1_d[:])
            nc.sync.dma_start(out=VA[:, 2 * WVA:], in_=va23_d[:])

            # ---- ACT claims: zero bias column + exp-table preload -------
            dume = epool.tile([1, 1], f32, tag="dume")
            nc.scalar.copy(dume, ZB[0:1, :])
            dume2 = epool.tile([1, 1], f32, tag="dume2")
            nc.scalar.activation(dume2, ZB[0:1, :],
                                 mybir.ActivationFunctionType.Exp,
                                 bias=ZB[0:1, :])

            # ---- DVE: claim pair-0 kq pieces, then replicate aug rows ---
            dumv = epool.tile([1, 1], bf16, tag="dumv")
            if DEDUP:
                nc.vector.tensor_copy(dumv, KQ[64:65, P:P + 1])            # kq0a
                nc.vector.tensor_copy(dumv, KQ[64:65, SPLIT0:SPLIT0 + 1])  # kq0b
                nc.vector.nop(nofuse=True)
                for j in range(1, PAIRS):
                    nc.vector.nop(nofuse=True)
                    nc.vector.tensor_copy(
                        KQ[64:, j * WKQ + P:(j + 1) * WKQ],
                        KQ[64:, P:WKQ])

            GL = [GROUPS] * PAIRS
            state = {}

            def claim(engine_matmul_src):
                nc.tensor.matmul(dumm_slot(), lhsT=engine_matmul_src,
                                 rhs=engine_matmul_src,
                                 start=True, stop=True, skip_group_check=True)

            def scores_group(j, gi):
                st = state.setdefault(j, {})
                c0, ng = GL[j][gi]
                w = j * WKQ
                qhat = KQ[:, w:w + P]
                if j == 0:
                    # pair 0 lands in two pieces; claim each as it is used
                    if gi == 0:
                        claim(KQ[0:1, 0:1])
                    elif gi == 1:
                        claim(KQ[0:1, SPLIT0:SPLIT0 + 1])
                elif gi == 0:
                    claim(KQ[0:1, w:w + 1])                # per-head kq DMA
                    if DEDUP:
                        if j == 1:
                            claim(KQ[64:65, w:w + 1])      # qaugr DMA
                        claim(KQ[64:65, w + P:w + P + 1])  # DVE replica j
                # dep-free donor nops: _repair_multi_waits parks displaced
                # waits here, adjacent to the instructions that need them
                nc.tensor.nop(nofuse=True)
                nc.tensor.nop(nofuse=True)
                sp = (sp8sum.tile([128, 1024], f32, name="sp8") if ng > 1
                      else sp1sum.tile([128, 128], f32, name="sp1"))
                for i in range(ng):
                    c = c0 + i
                    nc.tensor.matmul(
                        sp[:, i * P:(i + 1) * P],
                        lhsT=KQ[:, w + P + c * 128:w + P + (c + 1) * 128],
                        rhs=qhat,
                        start=True, stop=True)
                st.setdefault("sps", []).append(sp)

            def exp_group(j, gi):
                st = state[j]
                c0, ng = GL[j][gi]
                sp = st["sps"][gi]
                nc.scalar.activation(
                    PT[:, j * WPT + c0 * P:j * WPT + (c0 + ng) * P],
                    sp[:, :ng * P],
                    mybir.ActivationFunctionType.Exp, bias=ZB)

            def pv_group(j, gi):
                st = state[j]
                c0, ng = GL[j][gi]
                if gi == 0:
                    # claims: va DMA (lane changes at pairs 0,1,2), acc WAR
                    if j == 0:
                        claim(VA[0:1, 0:1])
                    elif j == 1:
                        claim(VA[0:1, WVA:WVA + 1])
                    elif j == 2:
                        claim(VA[0:1, 2 * WVA:2 * WVA + 1])
                    st["acc"] = apsum.tile([P, 65], f32, name="acc")
                    nc.tensor.nop(nofuse=True)
                    if j >= 2:
                        # absorb the WAR wait on the recycled acc buffer
                        nc.tensor.matmul(st["acc"][0:1, 64:65],
                                         lhsT=CB1, rhs=CB1,
                                         start=True, stop=True,
                                         skip_group_check=True)
                for i in range(ng):
                    c = c0 + i
                    nc.tensor.matmul(
                        st["acc"],
                        lhsT=PT[:, j * WPT + c * P:j * WPT + (c + 1) * P],
                        rhs=VA[:, j * WVA + c * 65:j * WVA + (c + 1) * 65],
                        start=(c == 0), stop=(c == NCHUNK - 1))

            def evac_pair(j):
                st = state[j]
                acc_sb = OS[:, j * 65:(j + 1) * 65]
                nc.vector.tensor_copy(acc_sb, st["acc"])
                nc.tensor.matmul(dumm if j == PAIRS - 1 else dumm_slot(),
                                 lhsT=CB1, rhs=CB1,
                                 start=True, stop=True,
                                 skip_group_check=True)
                # gpsimd claimer absorbs the DVE data wait so the SWDGE
                # dma_start itself carries no semaphore wait
                dumg = epool.tile([1, 1], f32, tag=f"dumg{j}", name=f"dumg{j}")
                nc.gpsimd.tensor_copy(out=dumg, in_=acc_sb[0:1, 0:1])
                nc.gpsimd.dma_start(out=out_d[j], in_=acc_sb)

            # software pipeline: scores of pair j+1 interleave with exp/PV of j
            for gi in range(len(GL[0])):
                scores_group(0, gi)
            for j in range(PAIRS):
                nxt = GL[j + 1] if j + 1 < PAIRS else []
                for gi in range(len(GL[j])):
                    exp_group(j, gi)
                    pv_group(j, gi)
                    if gi < len(nxt):
                        scores_group(j + 1, gi)
                evac_pair(j)

    _repair_multi_waits(nc)
    return nc


def _get_nc():
    if "nc" not in _COMPILED:
        _COMPILED["nc"] = _build_module()
    return _COMPILED["nc"]


def kernel(pool_q, pool_k, pool_v, x_q, x_k, x_v, bias_slopes, regions,
           t_mask, n_mask, max_n):
    from concourse.bass_utils import run_bass_kernel_spmd

    kqa, va = _host_prep(
        np.asarray(pool_q, np.float32), np.asarray(pool_k, np.float32),
        np.asarray(pool_v, np.float32), np.asarray(x_q, np.float32),
        np.asarray(x_k, np.float32), np.asarray(x_v, np.float32),
        np.asarray(bias_slopes, np.float32), np.asarray(regions))

    SPLIT0 = P + 8 * 128
    in_maps = []
    for c in range(NCORES):
        b, h0 = c // 4, 4 * (c % 4)
        kq = kqa[b]                                     # [H, 98, WKQ]
        kr = 64 if DEDUP else AUG
        m = {
            "kq0a": np.ascontiguousarray(kq[h0, :, :SPLIT0]),
            "kq0b": np.ascontiguousarray(kq[h0, :, SPLIT0:]),
            "kq1": np.ascontiguousarray(kq[h0 + 1, :kr]),
            "kq23": np.ascontiguousarray(
                np.swapaxes(kq[h0 + 2:h0 + 4, :kr], 0, 1).reshape(kr, 2 * WKQ)),
            "va0": np.ascontiguousarray(va[b, h0]),
            "va1": np.ascontiguousarray(va[b, h0 + 1]),
            "va23": np.ascontiguousarray(
                np.swapaxes(va[b, h0 + 2:h0 + 4], 0, 1).reshape(128, 2 * WVA)),
        }
        if DEDUP:
            m["qaugr"] = np.ascontiguousarray(
                np.swapaxes(kq[h0 + 1:h0 + 4, 64:, :P], 0, 1))
        in_maps.append(m)

    nc = _get_nc()
    res = run_bass_kernel_spmd(
        nc, in_maps, core_ids=list(range(NCORES)),
        trace=bool(int(os.environ.get("KERNEL_TRACE", "0"))))
    _COMPILED["last_result"] = res

    out = np.empty((B, H, P, 64), np.float32)
    for c in range(NCORES):
        b, h0 = c // 4, 4 * (c % 4)
        ot = res.results[c]["outt"]                        # [PAIRS, P, 65]
        out[b, h0:h0 + PAIRS] = ot[:, :, :64] / ot[:, :, 64:65]
    return out


# revision 57
# speedup vs baseline: 1.1328x; 1.0627x over previous
"""Trainium2 Bass kernel for nn_AttentionPoolDown.

Structure exploited:
  * reference returns out[:, :, :P, :] -- only the P=128 pool queries matter,
    attending over L = P + T = 2176 keys.
  * ALiBi-style bias -slope*|ridx_q - ridx_k| decomposes over integer region
    ids (0..32) as |a-b| = a + b - 2*sum_t 1[a>=t]*1[b>=t], so the entire
    logits tensor scale*QK^T + bias is ONE matmul with an augmented
    contraction dim of 98: [64 roped dims | 32 indicator dims | 1 | ridx].
  * scores are bounded (|logits| < ~40) so softmax needs no max-subtraction:
    p = exp(logits), out = (p @ V) / (p @ 1).  Appending a ones-column to V
    yields the row sums for free in the same PV matmul.
  * Everything is computed in transposed layout-B ([keys, queries] chunks of
    128) so no on-chip transposes are ever needed.  The PV matmul runs with
    p stationary and V moving (65 moving rows per chunk instead of 128) and
    lands the accumulator directly in the output's [q, d] layout.
  * The 34 augmented contraction rows on the key side (indicators/ones/ridx)
    are head-INDEPENDENT: they are DMA'd once per core and replicated to the
    other 3 head windows by the otherwise-idle DVE (4x bf16 copy mode),
    cutting HBM traffic by ~15%.
  * bf16 storage + matmuls (accumulation in fp32 PSUM); rel err ~6e-3.

Sharding: B*H = 32 (b,h) pairs, 4 per core; core c handles b = c//4,
heads 4*(c%4)..4*(c%4)+3.

The walrus build here rejects instructions carrying more than ONE semaphore
wait, and Tile converts any same-engine data dependency into a "wait for all
prior own-engine instructions" self-wait.  The structure below funnels every
instruction's dependencies through a single semaphore: dep-free warmups
absorb preamble-barrier ticks, tiny claimer ops absorb DMA/cross-engine
waits in program order (Tile elides the now-redundant waits on the real
consumers), PSUM tiles are evacuated through DVE only, all big SBUF tensors
are statically placed (no pool-rotation WAR), and the out-DMAs ride gpsimd
SWDGE queues (off the HWDGE semaphore pool).
"""

import os
import numpy as np
import ml_dtypes

B, H, D, T = 2, 16, 64, 2048
MAX_N, R = 32, 4
P = MAX_N * R           # 128 pool tokens (these are the queries)
L = P + T               # 2176 keys
THETA = 10000.0
SCALE = 1.0 / np.sqrt(D)
AUG = 98                # 64 + 32 + 2 augmented contraction
NCHUNK = L // 128       # 17 key chunks
NCORES = 8
PAIRS = (B * H) // NCORES   # 4 (b,h) pairs per core

WKQ = P + L             # 2304 cols per head window in the KQ tile
WVA = NCHUNK * 65       # 1105 cols per head window in the VA tile
WPT = NCHUNK * P        # 2176 cols per head window in the PT tile

_COMPILED = {}

# experiment toggles (defaults = the fastest HW-validated configuration)
SKIP_INIT_BARRIER = bool(int(os.environ.get("K_SKIP_INIT_BARRIER", "1")))
DEDUP = bool(int(os.environ.get("K_DEDUP", "0")))


def _rope_pair(x, pos):
    """x: [..., L, 32], pos: [..., L] -> rotary split-half, Dh=32."""
    inv = (1.0 / (THETA ** (np.arange(0, 32, dtype=np.float32)[::2] / 32.0))).astype(np.float32)
    ang = pos[..., :, None] * inv                       # [..., L, 16]
    c, s = np.cos(ang), np.sin(ang)
    x1, x2 = x[..., :16], x[..., 16:]
    return np.concatenate([x1 * c - x2 * s, x1 * s + x2 * c], axis=-1)


def _host_prep(pool_q, pool_k, pool_v, x_q, x_k, x_v, bias_slopes, regions):
    """Returns kqa [B,H,98,WKQ] bf16, va [B,H,128,WVA] bf16."""
    regions = regions.astype(np.int32)
    n_ids = np.arange(1, MAX_N + 1, dtype=np.int32)

    eq = regions[:, None, :] == n_ids[None, :, None]            # [B,32,T]
    starts = np.argmax(eq, axis=-1).astype(np.float32)          # [B,32]
    pool_gpos = (starts[..., None] + 0.5 * np.arange(R, dtype=np.float32)).reshape(B, P)
    gpos = np.concatenate(
        [pool_gpos, np.broadcast_to(np.arange(T, dtype=np.float32), (B, T))], -1)
    pool_ridx = np.broadcast_to(np.repeat(n_ids, R), (B, P))
    ridx = np.concatenate([pool_ridx, regions], -1).astype(np.float32)   # [B,L]

    k = np.concatenate([pool_k, x_k], axis=2)                   # [B,H,L,64]
    gpos_b = gpos[:, None]                                      # [B,1,L]
    ridx_b = ridx[:, None]
    kr = np.concatenate(
        [_rope_pair(k[..., :32], gpos_b), _rope_pair(k[..., 32:], ridx_b)], -1)
    qr = np.concatenate(
        [_rope_pair(pool_q[..., :32], gpos_b[..., :P]),
         _rope_pair(pool_q[..., 32:], ridx_b[..., :P])], -1)    # [B,H,P,64]

    Bind = (ridx[:, None, :] >= n_ids[:, None].astype(np.float32)).astype(np.float32)  # [B,32,L]
    sl = bias_slopes.astype(np.float32)                         # [H]

    kqa = np.empty((B, H, AUG, WKQ), np.float32)
    kqa[:, :, :64, P:] = np.swapaxes(kr, -1, -2)
    kqa[:, :, 64:96, P:] = Bind[:, None]
    kqa[:, :, 96, P:] = 1.0
    kqa[:, :, 97, P:] = ridx[:, None]
    kqa[:, :, :64, :P] = SCALE * np.swapaxes(qr, -1, -2)
    kqa[:, :, 64:96, :P] = 2.0 * sl[None, :, None, None] * Bind[:, None, :, :P]
    kqa[:, :, 96, :P] = -sl[None, :, None] * ridx[:, None, :P]
    kqa[:, :, 97, :P] = -sl[None, :, None]

    v = np.concatenate([pool_v, x_v], axis=2)                   # [B,H,L,64]
    vaug = np.concatenate([v, np.ones((B, H, L, 1), np.float32)], -1)
    va = vaug.reshape(B, H, NCHUNK, 128, 65).transpose(0, 1, 3, 2, 4).reshape(
        B, H, 128, WVA)                                         # [B,H,128,WVA]
    return kqa.astype(ml_dtypes.bfloat16), va.astype(ml_dtypes.bfloat16)


def _patch_tile_drain():
    """The walrus build in this container rejects instructions with more than
    one semaphore wait.  Tile's kernel-tail drain aggregates the whole vector
    clock onto a single Drain -- split those waits across preceding
    single-wait sync-engine nops."""
    import bass_rust
    import concourse.tile as tile
    from concourse.vector_clock import ScopedClock
    if getattr(tile.TileContext, "_drain_split_patched", False):
        return

    def patched(self, tick_clock, wait_clock):
        # The wait-walk nops ride the otherwise-idle Pool engine so they can
        # burn through already-satisfied sems while SP is still occupied
        # issuing the tail out-DMAs; the closing all-engine barrier makes the
        # drain sound even though the drain itself keeps only one wait.
        nc = self.nc
        nops = [nc.sync.nop(nofuse=True) for _ in range(17)]
        drain_inst = nc.sync.drain()
        wait_clock.add_sem_waits(
            drain_inst.ins, ScopedClock({None: tick_clock.global_clock}))
        si = drain_inst.ins.sync_info
        waits = list(si.on_wait) if si is not None else []
        # order the walk by expected fire time (late sems last) so one
        # late-firing semaphore doesn't serialize the remaining waits
        prio = getattr(nc, "_drain_wait_prio", {})
        waits.sort(key=lambda w: prio.get(
            (w.ant_name or "").rsplit("_", 1)[0], 50))
        if len(waits) > 1:
            upd = list(si.on_update)
            assert len(waits) - 1 <= len(nops)
            for nop, w in zip(nops, waits[:-1]):
                old = nop.ins.sync_info
                nupd = list(old.on_update) if old is not None else []
                nop.ins.sync_info = bass_rust.SyncInfo(
                    on_wait=[w], on_update=nupd)
            drain_inst.ins.sync_info = bass_rust.SyncInfo(
                on_wait=[waits[-1]], on_update=upd)
        nc.all_engine_barrier()
        assert self.sems is not None
        popped = nc._tile_sem_poison_stack.pop()
        assert popped is self._sem_poison
        nc.clear_and_free_semaphores(list(self.sems.allocated().values()))
        nc.all_engine_barrier()

    tile.TileContext._drain_and_barrier = patched
    tile.TileContext._drain_split_patched = True


def _patch_skip_init_barrier():
    """Bass.__init__ ends with an all-engine barrier guarding the const-AP
    memsets.  This kernel never reads the const pool (every activation bias
    is an explicit AP), and the ~1us barrier sits directly on the
    first-input-DMA critical path -- skip just that one barrier."""
    import concourse.bass as bass
    if getattr(bass.Bass, "_init_barrier_skip_patched", False):
        return
    orig = bass.Bass.all_engine_barrier

    def patched(self, *, sem_only: bool = False):
        if not getattr(self, "_init_barrier_skipped", False):
            self._init_barrier_skipped = True
            return
        return orig(self, sem_only=sem_only)

    bass.Bass.all_engine_barrier = patched
    bass.Bass._init_barrier_skip_patched = True


def _repair_multi_waits(nc):
    """Walrus rejects instructions with more than one semaphore wait.  Tile's
    scheduler occasionally lands 2-3 waits on one instruction (its internal
    ordering displaces the hand-written claimer ops).  Repair post-hoc: move
    all but one wait of each offender onto the nearest PRECEDING wait-free
    instructions of the same engine.  Moving a wait earlier on the same
    engine only strengthens ordering; cycle-freedom is re-checked by the
    timeline simulation after the build."""
    import bass_rust

    DONOR_TYPES = (
        "InstMatmult", "InstNoOp", "InstTensorCopy", "InstActivation",
        "InstMemset", "InstTensorTensor", "InstTensorScalarPtr",
    )
    f = nc.m.functions[0]
    moved = []
    for blk in f.blocks:
        insts = list(blk.instructions)
        donors = {}                      # engine -> wait-free donor stack
        for i in insts:
            eng = i.engine.name
            si = getattr(i, "sync_info", None)
            ws = list(si.on_wait) if si is not None else []
            if len(ws) <= 1:
                if (not ws) and type(i).__name__ in DONOR_TYPES:
                    donors.setdefault(eng, []).append(i)
                continue
            dl = donors.setdefault(eng, [])

            def is_dma(w):
                return (w.ant_name or "").startswith(("DMAHW", "DMASW"))

            def is_self(w):
                return (w.ant_name or "").startswith(eng + "_")

            # 1. PE/DVE run their pipes strictly in order, so a self-wait on
            #    an instruction of the same engine is redundant: drop it.
            if eng in ("PE", "DVE"):
                dropped = [w for w in ws if is_self(w)]
                ws = [w for w in ws if not is_self(w)]
                for w in dropped:
                    moved.append((i.name, "<dropped>", w.ant_name,
                                  w.wait_value, "self"))
            # 2. DMA-lane waits are position-safe (input DMAs fire
            #    unconditionally): park them on any preceding donor.
            engws = [w for w in ws if not is_dma(w)]
            dmaws = [w for w in ws if is_dma(w)]
            while len(engws) + len(dmaws) > 1 and dmaws and dl:
                d = dl.pop()
                dsi = d.sync_info
                dupd = list(dsi.on_update) if dsi is not None else []
                w = dmaws.pop()
                d.sync_info = bass_rust.SyncInfo(on_wait=[w], on_update=dupd)
                moved.append((i.name, d.name, w.ant_name, w.wait_value, "dma"))
            keep = engws + dmaws
            assert len(keep) <= 1, (
                f"_repair_multi_waits: {i.name} ({eng}) still needs "
                f"{[str(w) for w in keep]}; engine-wait moves are unsafe "
                f"(they can cycle on in-order sequencers) -- restructure")
            upd = list(si.on_update)
            i.sync_info = bass_rust.SyncInfo(on_wait=keep, on_update=upd)
    return moved


def _build_module():
    import concourse.bass as bass
    import concourse.tile as tile
    from concourse import mybir

    _patch_tile_drain()
    if SKIP_INIT_BARRIER:
        _patch_skip_init_barrier()

    f32 = mybir.dt.float32
    bf16 = mybir.dt.bfloat16
    nc = bass.Bass(num_swdge_queues=2)

    # ---- DRAM parameters (per core) -------------------------------------
    # pair 0's kq window arrives in two pieces so the first score matmuls
    # start before the whole tensor lands; pairs 1-3 rows 0..63 are per-head
    # unique, rows 64..97 of the key part are replicated on-chip.
    kq0a_d = nc.declare_dram_parameter("kq0a", [AUG, P + 8 * 128], bf16, isOutput=False)
    kq0b_d = nc.declare_dram_parameter("kq0b", [AUG, 9 * 128], bf16, isOutput=False)
    KROWS = 64 if DEDUP else AUG
    kq1_d = nc.declare_dram_parameter("kq1", [KROWS, WKQ], bf16, isOutput=False)
    if DEDUP:
        kq23_d = nc.declare_dram_parameter("kq23", [KROWS, 2 * WKQ], bf16, isOutput=False)
    else:
        kq2_d = nc.declare_dram_parameter("kq2", [KROWS, WKQ], bf16, isOutput=False)
        kq3_d = nc.declare_dram_parameter("kq3", [KROWS, WKQ], bf16, isOutput=False)
    qaugr_d = (nc.declare_dram_parameter("qaugr", [34, 3, P], bf16, isOutput=False)
               if DEDUP else None)
    va0_d = nc.declare_dram_parameter("va0", [128, WVA], bf16, isOutput=False)
    va1_d = nc.declare_dram_parameter("va1", [128, WVA], bf16, isOutput=False)
    va23_d = nc.declare_dram_parameter("va23", [128, 2 * WVA], bf16, isOutput=False)
    out_d = nc.declare_dram_parameter("outt", [PAIRS, P, 65], f32, isOutput=True)

    GROUPS = [(0, 8), (8, 8), (16, 1)]
    SPLIT0 = P + 8 * 128        # pair-0 kq piece boundary

    # expected semaphore fire order for the drain walk: HWDGE lanes 0..7 are
    # the input DMAs in issue order (all fire mid-kernel); the SWDGE lanes
    # carry the out-DMAs, of which the last two fire after the compute tail.
    nc._drain_wait_prio = {
        "DMAHW0": 0, "DMAHW1": 1, "DMAHW2": 2, "DMAHW3": 3,
        "DMAHW4": 4, "DMAHW5": 5, "DMAHW6": 6, "DMAHW7": 7,
        "Pool": 20, "Activation": 21, "PE": 22, "DVE": 23,
        "DMASW0": 30, "DMASW1": 31, "DMASW2": 32, "DMASW3": 33,
        "DMASW4": 34, "DMASW5": 35, "DMASW6": 36, "DMASW7": 37,
    }

    with tile.TileContext(nc) as tc:
        with (
            tc.tile_pool(name="kq", bufs=1) as kqpool,
            tc.tile_pool(name="va", bufs=1) as vapool,
            tc.tile_pool(name="pt", bufs=1) as ptpool,
            tc.tile_pool(name="os", bufs=1) as ospool,
            tc.tile_pool(name="epool", bufs=4) as epool,
            tc.tile_pool(name="sp8sum", bufs=2, space="PSUM") as sp8sum,
            tc.tile_pool(name="sp1sum", bufs=1, space="PSUM") as sp1sum,
            tc.tile_pool(name="apsum", bufs=2, space="PSUM") as apsum,
            tc.tile_pool(name="dpsum", bufs=1, space="PSUM") as dpsum,
        ):
            # ---- static SBUF tensors ------------------------------------
            KQ = kqpool.tile([AUG, PAIRS * WKQ], bf16, name="KQ")
            VA = vapool.tile([128, PAIRS * WVA], bf16, name="VA")
            PT = ptpool.tile([128, PAIRS * WPT], bf16, name="PT")
            OS = ospool.tile([128, PAIRS * 65], f32, name="OS")

            # ---- dep-free warmups ---------------------------------------
            # dummy-claim matmuls each write their OWN psum byte: a shared
            # target would WAW-chain the claims, letting one late-firing
            # claim displace later ones behind their real consumers in
            # Tile's readiness-ordered schedule.
            dumm_all = dpsum.tile([1, 64], f32)
            dumm_ctr = [0]

            def dumm_slot():
                i = dumm_ctr[0]
                dumm_ctr[0] += 1
                return dumm_all[0:1, i:i + 1]

            dumm = dumm_slot()
            # dummy operand: the framework's const pool is initialized in the
            # preamble (before the entry barrier), so reads are dep-free AND
            # defined -- CoreSim rejects reads of uninitialized SBUF
            CB1 = nc.const_aps.aps[(bf16, 1.0)][0:1, 0:1]
            dwarm0 = epool.tile([1, 1], bf16, tag="dwarm")
            nc.tensor.matmul(dumm, lhsT=CB1, rhs=CB1,
                             start=True, stop=True, skip_group_check=True)
            nc.scalar.copy(dwarm0, CB1)
            # DVE warmup doubles as the exp-bias zero column
            ZB = epool.tile([128, 1], bf16, tag="zb")
            nc.vector.memset(ZB, 0.0)

            # ---- input DMAs (SP -> HWDGE), urgency order ----------------
            # exactly 8: one per HWDGE semaphore lane (a 9th would carry a
            # lane-FIFO wait on top of its data wait -> illegal on walrus)
            nc.sync.dma_start(out=KQ[:, :SPLIT0], in_=kq0a_d[:])
            nc.sync.dma_start(out=KQ[:, SPLIT0:WKQ], in_=kq0b_d[:])
            nc.sync.dma_start(out=KQ[:KROWS, WKQ:2 * WKQ], in_=kq1_d[:])
            if DEDUP:
                qaug_dst = KQ[64:, WKQ:].rearrange(
                    "p (w c) -> p w c", c=WKQ)[:, :, :P]
                nc.sync.dma_start(out=qaug_dst, in_=qaugr_d[:])
                nc.sync.dma_start(out=VA[:, :WVA], in_=va0_d[:])
                nc.sync.dma_start(out=KQ[:KROWS, 2 * WKQ:], in_=kq23_d[:])
                nc.sync.dma_start(out=VA[:, WVA:2 * WVA], in_=va1_d[:])
                nc.sync.dma_start(out=VA[:, 2 * WVA:], in_=va23_d[:])
            else:
                dmas = {
                    "va0": lambda: nc.sync.dma_start(out=VA[:, :WVA], in_=va0_d[:]),
                    "kq2": lambda: nc.sync.dma_start(out=KQ[:KROWS, 2 * WKQ:3 * WKQ], in_=kq2_d[:]),
                    "kq3": lambda: nc.sync.dma_start(out=KQ[:KROWS, 3 * WKQ:], in_=kq3_d[:]),
                    "va1": lambda: nc.sync.dma_start(out=VA[:, WVA:2 * WVA], in_=va1_d[:]),
                    "va23": lambda: nc.sync.dma_start(out=VA[:, 2 * WVA:], in_=va23_d[:]),
                }
                order = os.environ.get("K_DMA_ORDER", "va0,kq2,va1,kq3,va23")
                for name in order.split(","):
                    dmas[name]()

            # ---- ACT claims: zero bias column + exp-table preload -------
            dume = epool.tile([1, 1], f32, tag="dume")
            nc.scalar.copy(dume, ZB[0:1, :])
            dume2 = epool.tile([1, 1], f32, tag="dume2")
            nc.scalar.activation(dume2, ZB[0:1, :],
                                 mybir.ActivationFunctionType.Exp,
                                 bias=ZB[0:1, :])

            # ---- DVE: claim pair-0 kq pieces, then replicate aug rows ---
            dumv = epool.tile([1, 2], bf16, tag="dumv")
            if DEDUP:
                nc.vector.tensor_copy(dumv[:, 0:1], KQ[64:65, P:P + 1])            # kq0a
                nc.vector.tensor_copy(dumv[:, 1:2], KQ[64:65, SPLIT0:SPLIT0 + 1])  # kq0b
                nc.vector.nop(nofuse=True)
                for j in range(1, PAIRS):
                    nc.vector.nop(nofuse=True)
                    nc.vector.tensor_copy(
                        KQ[64:, j * WKQ + P:(j + 1) * WKQ],
                        KQ[64:, P:WKQ])

            GL = [GROUPS] * PAIRS
            state = {}

            def claim(engine_matmul_src):
                nc.tensor.matmul(dumm_slot(), lhsT=engine_matmul_src,
                                 rhs=engine_matmul_src,
                                 start=True, stop=True, skip_group_check=True)

            def scores_group(j, gi):
                st = state.setdefault(j, {})
                c0, ng = GL[j][gi]
                w = j * WKQ
                qhat = KQ[:, w:w + P]
                if j == 0:
                    # pair 0 lands in two pieces; claim each as it is used
                    if gi == 0:
                        claim(KQ[0:1, 0:1])
                    elif gi == 1:
                        claim(KQ[0:1, SPLIT0:SPLIT0 + 1])
                elif gi == 0:
                    claim(KQ[0:1, w:w + 1])                # per-head kq DMA
                    if DEDUP:
                        if j == 1:
                            claim(KQ[64:65, w:w + 1])      # qaugr DMA
                        claim(KQ[64:65, w + P:w + P + 1])  # DVE replica j
                # dep-free donor nops: _repair_multi_waits parks displaced
                # waits here, adjacent to the instructions that need them
                nc.tensor.nop(nofuse=True)
                nc.tensor.nop(nofuse=True)
                sp = (sp8sum.tile([128, 1024], f32, name="sp8") if ng > 1
                      else sp1sum.tile([128, 128], f32, name="sp1"))
                for i in range(ng):
                    c = c0 + i
                    nc.tensor.matmul(
                        sp[:, i * P:(i + 1) * P],
                        lhsT=KQ[:, w + P + c * 128:w + P + (c + 1) * 128],
                        rhs=qhat,
                        start=True, stop=True)
                st.setdefault("sps", []).append(sp)

            def exp_group(j, gi):
                st = state[j]
                c0, ng = GL[j][gi]
                sp = st["sps"][gi]
                nc.scalar.activation(
                    PT[:, j * WPT + c0 * P:j * WPT + (c0 + ng) * P],
                    sp[:, :ng * P],
                    mybir.ActivationFunctionType.Exp, bias=ZB)

            def pv_group(j, gi):
                st = state[j]
                c0, ng = GL[j][gi]
                if gi == 0:
                    # claims: va DMA (lane changes at pairs 0,1,2), acc WAR
                    if j == 0:
                        claim(VA[0:1, 0:1])
                    elif j == 1:
                        claim(VA[0:1, WVA:WVA + 1])
                    elif j == 2:
                        claim(VA[0:1, 2 * WVA:2 * WVA + 1])
                    st["acc"] = apsum.tile([P, 65], f32, name="acc")
                    nc.tensor.nop(nofuse=True)
                    if j >= 2:
                        # absorb the WAR wait on the recycled acc buffer
                        nc.tensor.matmul(st["acc"][0:1, 64:65],
                                         lhsT=CB1, rhs=CB1,
                                         start=True, stop=True,
                                         skip_group_check=True)
                for i in range(ng):
                    c = c0 + i
                    nc.tensor.matmul(
                        st["acc"],
                        lhsT=PT[:, j * WPT + c * P:j * WPT + (c + 1) * P],
                        rhs=VA[:, j * WVA + c * 65:j * WVA + (c + 1) * 65],
                        start=(c == 0), stop=(c == NCHUNK - 1))

            def evac_pair(j):
                st = state[j]
                acc_sb = OS[:, j * 65:(j + 1) * 65]
                nc.vector.tensor_copy(acc_sb, st["acc"])
                nc.tensor.matmul(dumm if j == PAIRS - 1 else dumm_slot(),
                                 lhsT=CB1, rhs=CB1,
                                 start=True, stop=True,
                                 skip_group_check=True)
                # gpsimd claimer absorbs the DVE data wait so the SWDGE
                # dma_start itself carries no semaphore wait
                dumg = epool.tile([1, 1], f32, tag=f"dumg{j}", name=f"dumg{j}")
                nc.gpsimd.tensor_copy(out=dumg, in_=acc_sb[0:1, 0:1])
                nc.gpsimd.dma_start(out=out_d[j], in_=acc_sb)

            # software pipeline: scores of pair j+1 interleave with exp/PV of j
            for gi in range(len(GL[0])):
                scores_group(0, gi)
            for j in range(PAIRS):
                nxt = GL[j + 1] if j + 1 < PAIRS else []
                for gi in range(len(GL[j])):
                    exp_group(j, gi)
                    pv_group(j, gi)
                    if gi < len(nxt):
                        scores_group(j + 1, gi)
                evac_pair(j)

    _repair_multi_waits(nc)
    return nc


def _get_nc():
    if "nc" not in _COMPILED:
        _COMPILED["nc"] = _build_module()
    return _COMPILED["nc"]


def kernel(pool_q, pool_k, pool_v, x_q, x_k, x_v, bias_slopes, regions,
           t_mask, n_mask, max_n):
    from concourse.bass_utils import run_bass_kernel_spmd

    kqa, va = _host_prep(
        np.asarray(pool_q, np.float32), np.asarray(pool_k, np.float32),
        np.asarray(pool_v, np.float32), np.asarray(x_q, np.float32),
        np.asarray(x_k, np.float32), np.asarray(x_v, np.float32),
        np.asarray(bias_slopes, np.float32), np.asarray(regions))

    SPLIT0 = P + 8 * 128
    in_maps = []
    for c in range(NCORES):
        b, h0 = c // 4, 4 * (c % 4)
        kq = kqa[b]                                     # [H, 98, WKQ]
        kr = 64 if DEDUP else AUG
        m = {
            "kq0a": np.ascontiguousarray(kq[h0, :, :SPLIT0]),
            "kq0b": np.ascontiguousarray(kq[h0, :, SPLIT0:]),
            "kq1": np.ascontiguousarray(kq[h0 + 1, :kr]),
        }
        if DEDUP:
            m["kq23"] = np.ascontiguousarray(
                np.swapaxes(kq[h0 + 2:h0 + 4, :kr], 0, 1).reshape(kr, 2 * WKQ))
        else:
            m["kq2"] = np.ascontiguousarray(kq[h0 + 2, :kr])
            m["kq3"] = np.ascontiguousarray(kq[h0 + 3, :kr])
        m.update({
            "va0": np.ascontiguousarray(va[b, h0]),
            "va1": np.ascontiguousarray(va[b, h0 + 1]),
            "va23": np.ascontiguousarray(
                np.swapaxes(va[b, h0 + 2:h0 + 4], 0, 1).reshape(128, 2 * WVA)),
        })
        if DEDUP:
            m["qaugr"] = np.ascontiguousarray(
                np.swapaxes(kq[h0 + 1:h0 + 4, 64:, :P], 0, 1))
        in_maps.append(m)

    nc = _get_nc()
    res = run_bass_kernel_spmd(
        nc, in_maps, core_ids=list(range(NCORES)),
        trace=bool(int(os.environ.get("KERNEL_TRACE", "0"))))
    _COMPILED["last_result"] = res

    out = np.empty((B, H, P, 64), np.float32)
    for c in range(NCORES):
        b, h0 = c // 4, 4 * (c % 4)
        ot = res.results[c]["outt"]                        # [PAIRS, P, 65]
        out[b, h0:h0 + PAIRS] = ot[:, :, :64] / ot[:, :, 64:65]
    return out


# revision 61
# speedup vs baseline: 1.1690x; 1.0320x over previous
"""Trainium2 Bass kernel for nn_AttentionPoolDown.

Structure exploited:
  * reference returns out[:, :, :P, :] -- only the P=128 pool queries matter,
    attending over L = P + T = 2176 keys.
  * ALiBi-style bias -slope*|ridx_q - ridx_k| decomposes over integer region
    ids (0..32) as |a-b| = a + b - 2*sum_t 1[a>=t]*1[b>=t], so the entire
    logits tensor scale*QK^T + bias is ONE matmul with an augmented
    contraction dim of 98: [64 roped dims | 32 indicator dims | 1 | ridx].
  * scores are bounded (|logits| < ~40) so softmax needs no max-subtraction:
    p = exp(logits), out = (p @ V) / (p @ 1).  Appending a ones-column to V
    yields the row sums for free in the same PV matmul.
  * Everything is computed in transposed layout-B ([keys, queries] chunks of
    128) so no on-chip transposes are ever needed.  The PV matmul runs with
    p stationary and V moving (65 moving rows per chunk instead of 128) and
    lands the accumulator directly in the output's [q, d] layout.
  * bf16 storage + matmuls (accumulation in fp32 PSUM); rel err ~6e-3.

Sharding: B*H = 32 (b,h) pairs, 4 per core; core c handles b = c//4,
heads 4*(c%4)..4*(c%4)+3.

Scheduling (all verified on hardware):
  * 8 batched input DMAs -- exactly one per HWDGE semaphore lane; a 9th
    in-flight input DMA would carry a lane-FIFO wait on top of its data
    wait, which this walrus build rejects (one semaphore wait per
    instruction).  Out-DMAs reuse lanes 0-3 late, their lane-FIFO waits
    parked on donor nops.
  * Bass's const-pool init barrier (~1us) is skipped: nothing reads the
    const pool before the first compute, and every dummy operand reads the
    const-bf16-1.0 AP which the preamble memsets early.
  * Tile's list scheduler can land 2-3 semaphore waits on one instruction
    (it reorders claimer ops past their consumers).  _repair_multi_waits
    post-processes the scheduled stream: PE/DVE self-waits are dropped
    (those pipes execute in order), DMA-lane waits are parked on preceding
    donor nops (input DMAs fire unconditionally, so any earlier position
    is safe).  Engine-to-engine waits are never moved -- that can deadlock
    the in-order sequencers on real silicon even when the simulators
    (4-deep bypass queues) pass.
  * The kernel-tail drain is split across single-wait nops ordered by
    expected semaphore fire time, so one late out-DMA doesn't serialize
    the walk.
  * All big SBUF tensors are statically placed (no pool-rotation WAR);
    PSUM accumulators are evacuated through the otherwise-idle DVE.
"""

import os
import numpy as np
import ml_dtypes

B, H, D, T = 2, 16, 64, 2048
MAX_N, R = 32, 4
P = MAX_N * R           # 128 pool tokens (these are the queries)
L = P + T               # 2176 keys
THETA = 10000.0
SCALE = 1.0 / np.sqrt(D)
AUG = 98                # 64 + 32 + 2 augmented contraction
NCHUNK = L // 128       # 17 key chunks
NCORES = 8
PAIRS = (B * H) // NCORES   # 4 (b,h) pairs per core

WKQ = P + L             # 2304 cols per head window in the KQ tile
WVA = NCHUNK * 65       # 1105 cols per head window in the VA tile
WPT = NCHUNK * P        # 2176 cols per head window in the PT tile

_COMPILED = {}

# experiment toggles (defaults = the fastest HW-validated configuration)
SKIP_INIT_BARRIER = bool(int(os.environ.get("K_SKIP_INIT_BARRIER", "1")))
DEDUP = bool(int(os.environ.get("K_DEDUP", "0")))


def _rope_pair(x, pos):
    """x: [..., L, 32], pos: [..., L] -> rotary split-half, Dh=32."""
    inv = (1.0 / (THETA ** (np.arange(0, 32, dtype=np.float32)[::2] / 32.0))).astype(np.float32)
    ang = pos[..., :, None] * inv                       # [..., L, 16]
    c, s = np.cos(ang), np.sin(ang)
    x1, x2 = x[..., :16], x[..., 16:]
    return np.concatenate([x1 * c - x2 * s, x1 * s + x2 * c], axis=-1)


def _host_prep(pool_q, pool_k, pool_v, x_q, x_k, x_v, bias_slopes, regions):
    """Returns kqa [B,H,98,WKQ] bf16, va [B,H,128,WVA] bf16."""
    regions = regions.astype(np.int32)
    n_ids = np.arange(1, MAX_N + 1, dtype=np.int32)

    eq = regions[:, None, :] == n_ids[None, :, None]            # [B,32,T]
    starts = np.argmax(eq, axis=-1).astype(np.float32)          # [B,32]
    pool_gpos = (starts[..., None] + 0.5 * np.arange(R, dtype=np.float32)).reshape(B, P)
    gpos = np.concatenate(
        [pool_gpos, np.broadcast_to(np.arange(T, dtype=np.float32), (B, T))], -1)
    pool_ridx = np.broadcast_to(np.repeat(n_ids, R), (B, P))
    ridx = np.concatenate([pool_ridx, regions], -1).astype(np.float32)   # [B,L]

    k = np.concatenate([pool_k, x_k], axis=2)                   # [B,H,L,64]
    gpos_b = gpos[:, None]                                      # [B,1,L]
    ridx_b = ridx[:, None]
    kr = np.concatenate(
        [_rope_pair(k[..., :32], gpos_b), _rope_pair(k[..., 32:], ridx_b)], -1)
    qr = np.concatenate(
        [_rope_pair(pool_q[..., :32], gpos_b[..., :P]),
         _rope_pair(pool_q[..., 32:], ridx_b[..., :P])], -1)    # [B,H,P,64]

    Bind = (ridx[:, None, :] >= n_ids[:, None].astype(np.float32)).astype(np.float32)  # [B,32,L]
    sl = bias_slopes.astype(np.float32)                         # [H]

    kqa = np.empty((B, H, AUG, WKQ), np.float32)
    kqa[:, :, :64, P:] = np.swapaxes(kr, -1, -2)
    kqa[:, :, 64:96, P:] = Bind[:, None]
    kqa[:, :, 96, P:] = 1.0
    kqa[:, :, 97, P:] = ridx[:, None]
    kqa[:, :, :64, :P] = SCALE * np.swapaxes(qr, -1, -2)
    kqa[:, :, 64:96, :P] = 2.0 * sl[None, :, None, None] * Bind[:, None, :, :P]
    kqa[:, :, 96, :P] = -sl[None, :, None] * ridx[:, None, :P]
    kqa[:, :, 97, :P] = -sl[None, :, None]

    v = np.concatenate([pool_v, x_v], axis=2)                   # [B,H,L,64]
    vaug = np.concatenate([v, np.ones((B, H, L, 1), np.float32)], -1)
    va = vaug.reshape(B, H, NCHUNK, 128, 65).transpose(0, 1, 3, 2, 4).reshape(
        B, H, 128, WVA)                                         # [B,H,128,WVA]
    return kqa.astype(ml_dtypes.bfloat16), va.astype(ml_dtypes.bfloat16)


def _patch_tile_drain():
    """The walrus build in this container rejects instructions with more than
    one semaphore wait.  Tile's kernel-tail drain aggregates the whole vector
    clock onto a single Drain -- split those waits across preceding
    single-wait sync-engine nops."""
    import bass_rust
    import concourse.tile as tile
    from concourse.vector_clock import ScopedClock
    if getattr(tile.TileContext, "_drain_split_patched", False):
        return

    def patched(self, tick_clock, wait_clock):
        # The wait-walk nops ride the otherwise-idle Pool engine so they can
        # burn through already-satisfied sems while SP is still occupied
        # issuing the tail out-DMAs; the closing all-engine barrier makes the
        # drain sound even though the drain itself keeps only one wait.
        nc = self.nc
        nops = [nc.sync.nop(nofuse=True) for _ in range(17)]
        drain_inst = nc.sync.drain()
        wait_clock.add_sem_waits(
            drain_inst.ins, ScopedClock({None: tick_clock.global_clock}))
        si = drain_inst.ins.sync_info
        waits = list(si.on_wait) if si is not None else []
        # order the walk by expected fire time (late sems last) so one
        # late-firing semaphore doesn't serialize the remaining waits
        prio = getattr(nc, "_drain_wait_prio", {})
        waits.sort(key=lambda w: prio.get(
            (w.ant_name or "").rsplit("_", 1)[0], 50))
        if len(waits) > 1:
            upd = list(si.on_update)
            assert len(waits) - 1 <= len(nops)
            for nop, w in zip(nops, waits[:-1]):
                old = nop.ins.sync_info
                nupd = list(old.on_update) if old is not None else []
                nop.ins.sync_info = bass_rust.SyncInfo(
                    on_wait=[w], on_update=nupd)
            drain_inst.ins.sync_info = bass_rust.SyncInfo(
                on_wait=[waits[-1]], on_update=upd)
        nc.all_engine_barrier()
        assert self.sems is not None
        popped = nc._tile_sem_poison_stack.pop()
        assert popped is self._sem_poison
        nc.clear_and_free_semaphores(list(self.sems.allocated().values()))
        nc.all_engine_barrier()

    tile.TileContext._drain_and_barrier = patched
    tile.TileContext._drain_split_patched = True


def _patch_skip_init_barrier():
    """Bass.__init__ ends with an all-engine barrier guarding the const-AP
    memsets.  This kernel never reads the const pool (every activation bias
    is an explicit AP), and the ~1us barrier sits directly on the
    first-input-DMA critical path -- skip just that one barrier."""
    import concourse.bass as bass
    if getattr(bass.Bass, "_init_barrier_skip_patched", False):
        return
    orig = bass.Bass.all_engine_barrier

    def patched(self, *, sem_only: bool = False):
        if not getattr(self, "_init_barrier_skipped", False):
            self._init_barrier_skipped = True
            return
        return orig(self, sem_only=sem_only)

    bass.Bass.all_engine_barrier = patched
    bass.Bass._init_barrier_skip_patched = True


def _repair_multi_waits(nc):
    """Walrus rejects instructions with more than one semaphore wait.  Tile's
    scheduler occasionally lands 2-3 waits on one instruction (its internal
    ordering displaces the hand-written claimer ops).  Repair post-hoc: move
    all but one wait of each offender onto the nearest PRECEDING wait-free
    instructions of the same engine.  Moving a wait earlier on the same
    engine only strengthens ordering; cycle-freedom is re-checked by the
    timeline simulation after the build."""
    import bass_rust

    DONOR_TYPES = (
        "InstMatmult", "InstNoOp", "InstTensorCopy", "InstActivation",
        "InstMemset", "InstTensorTensor", "InstTensorScalarPtr",
    )
    f = nc.m.functions[0]
    moved = []
    for blk in f.blocks:
        insts = list(blk.instructions)
        donors = {}                      # engine -> wait-free donor stack
        for i in insts:
            eng = i.engine.name
            si = getattr(i, "sync_info", None)
            ws = list(si.on_wait) if si is not None else []
            if len(ws) <= 1:
                if (not ws) and type(i).__name__ in DONOR_TYPES:
                    donors.setdefault(eng, []).append(i)
                continue
            dl = donors.setdefault(eng, [])

            def is_dma(w):
                return (w.ant_name or "").startswith(("DMAHW", "DMASW"))

            def is_self(w):
                return (w.ant_name or "").startswith(eng + "_")

            # 1. PE/DVE run their pipes strictly in order, so a self-wait on
            #    an instruction of the same engine is redundant: drop it.
            if eng in ("PE", "DVE"):
                dropped = [w for w in ws if is_self(w)]
                ws = [w for w in ws if not is_self(w)]
                for w in dropped:
                    moved.append((i.name, "<dropped>", w.ant_name,
                                  w.wait_value, "self"))
            # 2. DMA-lane waits are position-safe (input DMAs fire
            #    unconditionally): park them on any preceding donor.
            engws = [w for w in ws if not is_dma(w)]
            dmaws = [w for w in ws if is_dma(w)]
            while len(engws) + len(dmaws) > 1 and dmaws and dl:
                d = dl.pop()
                dsi = d.sync_info
                dupd = list(dsi.on_update) if dsi is not None else []
                w = dmaws.pop()
                d.sync_info = bass_rust.SyncInfo(on_wait=[w], on_update=dupd)
                moved.append((i.name, d.name, w.ant_name, w.wait_value, "dma"))
            keep = engws + dmaws
            assert len(keep) <= 1, (
                f"_repair_multi_waits: {i.name} ({eng}) still needs "
                f"{[str(w) for w in keep]}; engine-wait moves are unsafe "
                f"(they can cycle on in-order sequencers) -- restructure")
            upd = list(si.on_update)
            i.sync_info = bass_rust.SyncInfo(on_wait=keep, on_update=upd)
    return moved


def _build_module():
    import concourse.bass as bass
    import concourse.tile as tile
    from concourse import mybir

    _patch_tile_drain()
    if SKIP_INIT_BARRIER:
        _patch_skip_init_barrier()

    f32 = mybir.dt.float32
    bf16 = mybir.dt.bfloat16
    nc = bass.Bass(num_swdge_queues=2)

    # ---- DRAM parameters (per core) -------------------------------------
    # pair 0's kq window arrives in two pieces so the first score matmuls
    # start before the whole tensor lands; pairs 1-3 rows 0..63 are per-head
    # unique, rows 64..97 of the key part are replicated on-chip.
    kq0a_d = nc.declare_dram_parameter("kq0a", [AUG, P + 8 * 128], bf16, isOutput=False)
    kq0b_d = nc.declare_dram_parameter("kq0b", [AUG, 9 * 128], bf16, isOutput=False)
    KROWS = 64 if DEDUP else AUG
    kq1_d = nc.declare_dram_parameter("kq1", [KROWS, WKQ], bf16, isOutput=False)
    if DEDUP:
        kq23_d = nc.declare_dram_parameter("kq23", [KROWS, 2 * WKQ], bf16, isOutput=False)
    else:
        kq2_d = nc.declare_dram_parameter("kq2", [KROWS, WKQ], bf16, isOutput=False)
        kq3_d = nc.declare_dram_parameter("kq3", [KROWS, WKQ], bf16, isOutput=False)
    qaugr_d = (nc.declare_dram_parameter("qaugr", [34, 3, P], bf16, isOutput=False)
               if DEDUP else None)
    va0_d = nc.declare_dram_parameter("va0", [128, WVA], bf16, isOutput=False)
    va1_d = nc.declare_dram_parameter("va1", [128, WVA], bf16, isOutput=False)
    va23_d = nc.declare_dram_parameter("va23", [128, 2 * WVA], bf16, isOutput=False)
    out_d = nc.declare_dram_parameter("outt", [PAIRS, P, 65], f32, isOutput=True)

    GROUPS = [(0, 8), (8, 8), (16, 1)]
    SPLIT0 = P + 8 * 128        # pair-0 kq piece boundary

    # expected semaphore fire order for the drain walk: HWDGE lanes 0..7 are
    # the input DMAs in issue order (all fire mid-kernel); the SWDGE lanes
    # carry the out-DMAs, of which the last two fire after the compute tail.
    nc._drain_wait_prio = {
        "DMAHW4": 4, "DMAHW5": 5, "DMAHW6": 6, "DMAHW7": 7,
        "Pool": 20, "Activation": 21, "PE": 22, "DVE": 23,
        "DMAHW0": 30, "DMAHW1": 31, "DMAHW2": 32, "DMAHW3": 33,
    }

    with tile.TileContext(nc) as tc:
        with (
            tc.tile_pool(name="kq", bufs=1) as kqpool,
            tc.tile_pool(name="va", bufs=1) as vapool,
            tc.tile_pool(name="pt", bufs=1) as ptpool,
            tc.tile_pool(name="os", bufs=1) as ospool,
            tc.tile_pool(name="epool", bufs=4) as epool,
            tc.tile_pool(name="sp8sum", bufs=2, space="PSUM") as sp8sum,
            tc.tile_pool(name="sp1sum", bufs=1, space="PSUM") as sp1sum,
            tc.tile_pool(name="apsum", bufs=2, space="PSUM") as apsum,
            tc.tile_pool(name="dpsum", bufs=1, space="PSUM") as dpsum,
        ):
            # ---- static SBUF tensors ------------------------------------
            KQ = kqpool.tile([AUG, PAIRS * WKQ], bf16, name="KQ")
            VA = vapool.tile([128, PAIRS * WVA], bf16, name="VA")
            PT = ptpool.tile([128, PAIRS * WPT], bf16, name="PT")
            OS = ospool.tile([128, PAIRS * 65], f32, name="OS")

            # ---- dep-free warmups ---------------------------------------
            # dummy-claim matmuls each write their OWN psum byte: a shared
            # target would WAW-chain the claims, letting one late-firing
            # claim displace later ones behind their real consumers in
            # Tile's readiness-ordered schedule.
            dumm_all = dpsum.tile([1, 64], f32)
            dumm_ctr = [0]

            def dumm_slot():
                i = dumm_ctr[0]
                dumm_ctr[0] += 1
                return dumm_all[0:1, i:i + 1]

            dumm = dumm_slot()
            # dummy operand: the framework's const pool is initialized in the
            # preamble (before the entry barrier), so reads are dep-free AND
            # defined -- CoreSim rejects reads of uninitialized SBUF
            CB1 = nc.const_aps.aps[(bf16, 1.0)][0:1, 0:1]
            dwarm0 = epool.tile([1, 1], bf16, tag="dwarm")
            nc.tensor.matmul(dumm, lhsT=CB1, rhs=CB1,
                             start=True, stop=True, skip_group_check=True)
            nc.scalar.copy(dwarm0, CB1)
            # DVE warmup doubles as the exp-bias zero column
            ZB = epool.tile([128, 1], bf16, tag="zb")
            nc.vector.memset(ZB, 0.0)

            # ---- input DMAs (SP -> HWDGE), urgency order ----------------
            # exactly 8: one per HWDGE semaphore lane (a 9th would carry a
            # lane-FIFO wait on top of its data wait -> illegal on walrus)
            nc.sync.dma_start(out=KQ[:, :SPLIT0], in_=kq0a_d[:])
            nc.sync.dma_start(out=KQ[:, SPLIT0:WKQ], in_=kq0b_d[:])
            nc.sync.dma_start(out=KQ[:KROWS, WKQ:2 * WKQ], in_=kq1_d[:])
            if DEDUP:
                qaug_dst = KQ[64:, WKQ:].rearrange(
                    "p (w c) -> p w c", c=WKQ)[:, :, :P]
                nc.sync.dma_start(out=qaug_dst, in_=qaugr_d[:])
                nc.sync.dma_start(out=VA[:, :WVA], in_=va0_d[:])
                nc.sync.dma_start(out=KQ[:KROWS, 2 * WKQ:], in_=kq23_d[:])
                nc.sync.dma_start(out=VA[:, WVA:2 * WVA], in_=va1_d[:])
                nc.sync.dma_start(out=VA[:, 2 * WVA:], in_=va23_d[:])
            else:
                dmas = {
                    "va0": lambda: nc.sync.dma_start(out=VA[:, :WVA], in_=va0_d[:]),
                    "kq2": lambda: nc.sync.dma_start(out=KQ[:KROWS, 2 * WKQ:3 * WKQ], in_=kq2_d[:]),
                    "kq3": lambda: nc.sync.dma_start(out=KQ[:KROWS, 3 * WKQ:], in_=kq3_d[:]),
                    "va1": lambda: nc.sync.dma_start(out=VA[:, WVA:2 * WVA], in_=va1_d[:]),
                    "va23": lambda: nc.sync.dma_start(out=VA[:, 2 * WVA:], in_=va23_d[:]),
                }
                order = os.environ.get("K_DMA_ORDER", "va0,kq2,va1,kq3,va23")
                for name in order.split(","):
                    dmas[name]()

            # ---- ACT claims: zero bias column + exp-table preload -------
            dume = epool.tile([1, 1], f32, tag="dume")
            nc.scalar.copy(dume, ZB[0:1, :])
            dume2 = epool.tile([1, 1], f32, tag="dume2")
            nc.scalar.activation(dume2, ZB[0:1, :],
                                 mybir.ActivationFunctionType.Exp,
                                 bias=ZB[0:1, :])

            # ---- DVE: claim pair-0 kq pieces, then replicate aug rows ---
            dumv = epool.tile([1, 2], bf16, tag="dumv")
            if DEDUP:
                nc.vector.tensor_copy(dumv[:, 0:1], KQ[64:65, P:P + 1])            # kq0a
                nc.vector.tensor_copy(dumv[:, 1:2], KQ[64:65, SPLIT0:SPLIT0 + 1])  # kq0b
                nc.vector.nop(nofuse=True)
                for j in range(1, PAIRS):
                    nc.vector.nop(nofuse=True)
                    nc.vector.tensor_copy(
                        KQ[64:, j * WKQ + P:(j + 1) * WKQ],
                        KQ[64:, P:WKQ])

            GL = [GROUPS] * PAIRS
            state = {}

            def claim(engine_matmul_src):
                nc.tensor.matmul(dumm_slot(), lhsT=engine_matmul_src,
                                 rhs=engine_matmul_src,
                                 start=True, stop=True, skip_group_check=True)

            def scores_group(j, gi):
                st = state.setdefault(j, {})
                c0, ng = GL[j][gi]
                w = j * WKQ
                qhat = KQ[:, w:w + P]
                if j == 0:
                    # pair 0 lands in two pieces; claim each as it is used
                    if gi == 0:
                        claim(KQ[0:1, 0:1])
                    elif gi == 1:
                        claim(KQ[0:1, SPLIT0:SPLIT0 + 1])
                elif gi == 0:
                    claim(KQ[0:1, w:w + 1])                # per-head kq DMA
                    if DEDUP:
                        if j == 1:
                            claim(KQ[64:65, w:w + 1])      # qaugr DMA
                        claim(KQ[64:65, w + P:w + P + 1])  # DVE replica j
                # dep-free donor nops: _repair_multi_waits parks displaced
                # waits here, adjacent to the instructions that need them
                nc.tensor.nop(nofuse=True)
                nc.tensor.nop(nofuse=True)
                sp = (sp8sum.tile([128, 1024], f32, name="sp8") if ng > 1
                      else sp1sum.tile([128, 128], f32, name="sp1"))
                for i in range(ng):
                    c = c0 + i
                    nc.tensor.matmul(
                        sp[:, i * P:(i + 1) * P],
                        lhsT=KQ[:, w + P + c * 128:w + P + (c + 1) * 128],
                        rhs=qhat,
                        start=True, stop=True)
                st.setdefault("sps", []).append(sp)

            def exp_group(j, gi):
                st = state[j]
                c0, ng = GL[j][gi]
                sp = st["sps"][gi]
                nc.scalar.activation(
                    PT[:, j * WPT + c0 * P:j * WPT + (c0 + ng) * P],
                    sp[:, :ng * P],
                    mybir.ActivationFunctionType.Exp, bias=ZB)

            def pv_group(j, gi):
                st = state[j]
                c0, ng = GL[j][gi]
                if gi == 0:
                    # claims: va DMA (lane changes at pairs 0,1,2), acc WAR
                    if j == 0:
                        claim(VA[0:1, 0:1])
                    elif j == 1:
                        claim(VA[0:1, WVA:WVA + 1])
                    elif j == 2:
                        claim(VA[0:1, 2 * WVA:2 * WVA + 1])
                    st["acc"] = apsum.tile([P, 65], f32, name="acc")
                    nc.tensor.nop(nofuse=True)
                    if j >= 2:
                        # absorb the WAR wait on the recycled acc buffer
                        nc.tensor.matmul(st["acc"][0:1, 64:65],
                                         lhsT=CB1, rhs=CB1,
                                         start=True, stop=True,
                                         skip_group_check=True)
                for i in range(ng):
                    c = c0 + i
                    nc.tensor.matmul(
                        st["acc"],
                        lhsT=PT[:, j * WPT + c * P:j * WPT + (c + 1) * P],
                        rhs=VA[:, j * WVA + c * 65:j * WVA + (c + 1) * 65],
                        start=(c == 0), stop=(c == NCHUNK - 1))

            def evac_pair(j):
                st = state[j]
                acc_sb = OS[:, j * 65:(j + 1) * 65]
                nc.vector.tensor_copy(acc_sb, st["acc"])
                nc.tensor.matmul(dumm if j == PAIRS - 1 else dumm_slot(),
                                 lhsT=CB1, rhs=CB1,
                                 start=True, stop=True,
                                 skip_group_check=True)
                # out-DMA on a reused HWDGE lane: the lane-FIFO wait is
                # parked on the donor nop by _repair_multi_waits, leaving the
                # DMA with only its DVE data wait
                nc.sync.nop(nofuse=True)
                nc.sync.dma_start(out=out_d[j], in_=acc_sb)

            # software pipeline: scores of pair j+1 interleave with exp/PV of j
            for gi in range(len(GL[0])):
                scores_group(0, gi)
            for j in range(PAIRS):
                nxt = GL[j + 1] if j + 1 < PAIRS else []
                for gi in range(len(GL[j])):
                    exp_group(j, gi)
                    pv_group(j, gi)
                    if gi < len(nxt):
                        scores_group(j + 1, gi)
                evac_pair(j)

    _repair_multi_waits(nc)
    return nc


def _get_nc():
    if "nc" not in _COMPILED:
        _COMPILED["nc"] = _build_module()
    return _COMPILED["nc"]


def kernel(pool_q, pool_k, pool_v, x_q, x_k, x_v, bias_slopes, regions,
           t_mask, n_mask, max_n):
    from concourse.bass_utils import run_bass_kernel_spmd

    kqa, va = _host_prep(
        np.asarray(pool_q, np.float32), np.asarray(pool_k, np.float32),
        np.asarray(pool_v, np.float32), np.asarray(x_q, np.float32),
        np.asarray(x_k, np.float32), np.asarray(x_v, np.float32),
        np.asarray(bias_slopes, np.float32), np.asarray(regions))

    SPLIT0 = P + 8 * 128
    in_maps = []
    for c in range(NCORES):
        b, h0 = c // 4, 4 * (c % 4)
        kq = kqa[b]                                     # [H, 98, WKQ]
        kr = 64 if DEDUP else AUG
        m = {
            "kq0a": np.ascontiguousarray(kq[h0, :, :SPLIT0]),
            "kq0b": np.ascontiguousarray(kq[h0, :, SPLIT0:]),
            "kq1": np.ascontiguousarray(kq[h0 + 1, :kr]),
        }
        if DEDUP:
            m["kq23"] = np.ascontiguousarray(
                np.swapaxes(kq[h0 + 2:h0 + 4, :kr], 0, 1).reshape(kr, 2 * WKQ))
        else:
            m["kq2"] = np.ascontiguousarray(kq[h0 + 2, :kr])
            m["kq3"] = np.ascontiguousarray(kq[h0 + 3, :kr])
        m.update({
            "va0": np.ascontiguousarray(va[b, h0]),
            "va1": np.ascontiguousarray(va[b, h0 + 1]),
            "va23": np.ascontiguousarray(
                np.swapaxes(va[b, h0 + 2:h0 + 4], 0, 1).reshape(128, 2 * WVA)),
        })
        if DEDUP:
            m["qaugr"] = np.ascontiguousarray(
                np.swapaxes(kq[h0 + 1:h0 + 4, 64:, :P], 0, 1))
        in_maps.append(m)

    nc = _get_nc()
    res = run_bass_kernel_spmd(
        nc, in_maps, core_ids=list(range(NCORES)),
        trace=bool(int(os.environ.get("KERNEL_TRACE", "0"))))
    _COMPILED["last_result"] = res

    out = np.empty((B, H, P, 64), np.float32)
    for c in range(NCORES):
        b, h0 = c // 4, 4 * (c % 4)
        ot = res.results[c]["outt"]                        # [PAIRS, P, 65]
        out[b, h0:h0 + PAIRS] = ot[:, :, :64] / ot[:, :, 64:65]
    return out


# revision 62
# speedup vs baseline: 1.1872x; 1.0156x over previous
"""Trainium2 Bass kernel for nn_AttentionPoolDown.

Structure exploited:
  * reference returns out[:, :, :P, :] -- only the P=128 pool queries matter,
    attending over L = P + T = 2176 keys.
  * ALiBi-style bias -slope*|ridx_q - ridx_k| decomposes over integer region
    ids (0..32) as |a-b| = a + b - 2*sum_t 1[a>=t]*1[b>=t], so the entire
    logits tensor scale*QK^T + bias is ONE matmul with an augmented
    contraction dim of 98: [64 roped dims | 32 indicator dims | 1 | ridx].
  * scores are bounded (|logits| < ~40) so softmax needs no max-subtraction:
    p = exp(logits), out = (p @ V) / (p @ 1).  Appending a ones-column to V
    yields the row sums for free in the same PV matmul.
  * Everything is computed in transposed layout-B ([keys, queries] chunks of
    128) so no on-chip transposes are ever needed.  The PV matmul runs with
    p stationary and V moving (65 moving rows per chunk instead of 128) and
    lands the accumulator directly in the output's [q, d] layout.
  * bf16 storage + matmuls (accumulation in fp32 PSUM); rel err ~6e-3.

Sharding: B*H = 32 (b,h) pairs, 4 per core; core c handles b = c//4,
heads 4*(c%4)..4*(c%4)+3.

Scheduling (all verified on hardware):
  * 8 batched input DMAs -- exactly one per HWDGE semaphore lane; a 9th
    in-flight input DMA would carry a lane-FIFO wait on top of its data
    wait, which this walrus build rejects (one semaphore wait per
    instruction).  Out-DMAs reuse lanes 0-3 late, their lane-FIFO waits
    parked on donor nops.
  * Bass's const-pool init barrier (~1us) is skipped: nothing reads the
    const pool before the first compute, and every dummy operand reads the
    const-bf16-1.0 AP which the preamble memsets early.
  * Tile's list scheduler can land 2-3 semaphore waits on one instruction
    (it reorders claimer ops past their consumers).  _repair_multi_waits
    post-processes the scheduled stream: PE/DVE self-waits are dropped
    (those pipes execute in order), DMA-lane waits are parked on preceding
    donor nops (input DMAs fire unconditionally, so any earlier position
    is safe).  Engine-to-engine waits are never moved -- that can deadlock
    the in-order sequencers on real silicon even when the simulators
    (4-deep bypass queues) pass.
  * The kernel-tail drain is split across single-wait nops ordered by
    expected semaphore fire time, so one late out-DMA doesn't serialize
    the walk.
  * All big SBUF tensors are statically placed (no pool-rotation WAR);
    PSUM accumulators are evacuated through the otherwise-idle DVE.
"""

import os
import numpy as np
import ml_dtypes

B, H, D, T = 2, 16, 64, 2048
MAX_N, R = 32, 4
P = MAX_N * R           # 128 pool tokens (these are the queries)
L = P + T               # 2176 keys
THETA = 10000.0
SCALE = 1.0 / np.sqrt(D)
AUG = 98                # 64 + 32 + 2 augmented contraction
NCHUNK = L // 128       # 17 key chunks
NCORES = 8
PAIRS = (B * H) // NCORES   # 4 (b,h) pairs per core

WKQ = P + L             # 2304 cols per head window in the KQ tile
WVA = NCHUNK * 65       # 1105 cols per head window in the VA tile
WPT = NCHUNK * P        # 2176 cols per head window in the PT tile

_COMPILED = {}

# experiment toggles (defaults = the fastest HW-validated configuration)
SKIP_INIT_BARRIER = bool(int(os.environ.get("K_SKIP_INIT_BARRIER", "1")))
DEDUP = bool(int(os.environ.get("K_DEDUP", "0")))


def _rope_pair(x, pos):
    """x: [..., L, 32], pos: [..., L] -> rotary split-half, Dh=32."""
    inv = (1.0 / (THETA ** (np.arange(0, 32, dtype=np.float32)[::2] / 32.0))).astype(np.float32)
    ang = pos[..., :, None] * inv                       # [..., L, 16]
    c, s = np.cos(ang), np.sin(ang)
    x1, x2 = x[..., :16], x[..., 16:]
    return np.concatenate([x1 * c - x2 * s, x1 * s + x2 * c], axis=-1)


def _host_prep(pool_q, pool_k, pool_v, x_q, x_k, x_v, bias_slopes, regions):
    """Returns kqa [B,H,98,WKQ] bf16, va [B,H,128,WVA] bf16."""
    regions = regions.astype(np.int32)
    n_ids = np.arange(1, MAX_N + 1, dtype=np.int32)

    eq = regions[:, None, :] == n_ids[None, :, None]            # [B,32,T]
    starts = np.argmax(eq, axis=-1).astype(np.float32)          # [B,32]
    pool_gpos = (starts[..., None] + 0.5 * np.arange(R, dtype=np.float32)).reshape(B, P)
    gpos = np.concatenate(
        [pool_gpos, np.broadcast_to(np.arange(T, dtype=np.float32), (B, T))], -1)
    pool_ridx = np.broadcast_to(np.repeat(n_ids, R), (B, P))
    ridx = np.concatenate([pool_ridx, regions], -1).astype(np.float32)   # [B,L]

    k = np.concatenate([pool_k, x_k], axis=2)                   # [B,H,L,64]
    gpos_b = gpos[:, None]                                      # [B,1,L]
    ridx_b = ridx[:, None]
    kr = np.concatenate(
        [_rope_pair(k[..., :32], gpos_b), _rope_pair(k[..., 32:], ridx_b)], -1)
    qr = np.concatenate(
        [_rope_pair(pool_q[..., :32], gpos_b[..., :P]),
         _rope_pair(pool_q[..., 32:], ridx_b[..., :P])], -1)    # [B,H,P,64]

    Bind = (ridx[:, None, :] >= n_ids[:, None].astype(np.float32)).astype(np.float32)  # [B,32,L]
    sl = bias_slopes.astype(np.float32)                         # [H]

    kqa = np.empty((B, H, AUG, WKQ), np.float32)
    kqa[:, :, :64, P:] = np.swapaxes(kr, -1, -2)
    kqa[:, :, 64:96, P:] = Bind[:, None]
    kqa[:, :, 96, P:] = 1.0
    kqa[:, :, 97, P:] = ridx[:, None]
    kqa[:, :, :64, :P] = SCALE * np.swapaxes(qr, -1, -2)
    kqa[:, :, 64:96, :P] = 2.0 * sl[None, :, None, None] * Bind[:, None, :, :P]
    kqa[:, :, 96, :P] = -sl[None, :, None] * ridx[:, None, :P]
    kqa[:, :, 97, :P] = -sl[None, :, None]

    v = np.concatenate([pool_v, x_v], axis=2)                   # [B,H,L,64]
    vaug = np.concatenate([v, np.ones((B, H, L, 1), np.float32)], -1)
    va = vaug.reshape(B, H, NCHUNK, 128, 65).transpose(0, 1, 3, 2, 4).reshape(
        B, H, 128, WVA)                                         # [B,H,128,WVA]
    return kqa.astype(ml_dtypes.bfloat16), va.astype(ml_dtypes.bfloat16)


def _patch_tile_drain():
    """The walrus build in this container rejects instructions with more than
    one semaphore wait.  Tile's kernel-tail drain aggregates the whole vector
    clock onto a single Drain -- split those waits across preceding
    single-wait sync-engine nops."""
    import bass_rust
    import concourse.tile as tile
    from concourse.vector_clock import ScopedClock
    if getattr(tile.TileContext, "_drain_split_patched", False):
        return

    def patched(self, tick_clock, wait_clock):
        # The wait-walk nops ride the otherwise-idle Pool engine so they can
        # burn through already-satisfied sems while SP is still occupied
        # issuing the tail out-DMAs; the closing all-engine barrier makes the
        # drain sound even though the drain itself keeps only one wait.
        nc = self.nc
        nops = [nc.sync.nop(nofuse=True) for _ in range(17)]
        drain_inst = nc.sync.drain()
        wait_clock.add_sem_waits(
            drain_inst.ins, ScopedClock({None: tick_clock.global_clock}))
        si = drain_inst.ins.sync_info
        waits = list(si.on_wait) if si is not None else []
        # order the walk by expected fire time (late sems last) so one
        # late-firing semaphore doesn't serialize the remaining waits
        prio = getattr(nc, "_drain_wait_prio", {})
        waits.sort(key=lambda w: prio.get(
            (w.ant_name or "").rsplit("_", 1)[0], 50))
        if len(waits) > 1:
            upd = list(si.on_update)
            assert len(waits) - 1 <= len(nops)
            for nop, w in zip(nops, waits[:-1]):
                old = nop.ins.sync_info
                nupd = list(old.on_update) if old is not None else []
                nop.ins.sync_info = bass_rust.SyncInfo(
                    on_wait=[w], on_update=nupd)
            drain_inst.ins.sync_info = bass_rust.SyncInfo(
                on_wait=[waits[-1]], on_update=upd)
        nc.all_engine_barrier()
        assert self.sems is not None
        popped = nc._tile_sem_poison_stack.pop()
        assert popped is self._sem_poison
        nc.clear_and_free_semaphores(list(self.sems.allocated().values()))
        # no closing barrier: launch completion already requires the
        # clearing engine to halt, so the clears are ordered before any
        # subsequent launch without stalling the other engines here

    tile.TileContext._drain_and_barrier = patched
    tile.TileContext._drain_split_patched = True


def _patch_skip_init_barrier():
    """Bass.__init__ ends with an all-engine barrier guarding the const-AP
    memsets.  This kernel never reads the const pool (every activation bias
    is an explicit AP), and the ~1us barrier sits directly on the
    first-input-DMA critical path -- skip just that one barrier."""
    import concourse.bass as bass
    if getattr(bass.Bass, "_init_barrier_skip_patched", False):
        return
    orig = bass.Bass.all_engine_barrier

    def patched(self, *, sem_only: bool = False):
        if not getattr(self, "_init_barrier_skipped", False):
            self._init_barrier_skipped = True
            return
        return orig(self, sem_only=sem_only)

    bass.Bass.all_engine_barrier = patched
    bass.Bass._init_barrier_skip_patched = True


def _repair_multi_waits(nc):
    """Walrus rejects instructions with more than one semaphore wait.  Tile's
    scheduler occasionally lands 2-3 waits on one instruction (its internal
    ordering displaces the hand-written claimer ops).  Repair post-hoc: move
    all but one wait of each offender onto the nearest PRECEDING wait-free
    instructions of the same engine.  Moving a wait earlier on the same
    engine only strengthens ordering; cycle-freedom is re-checked by the
    timeline simulation after the build."""
    import bass_rust

    DONOR_TYPES = (
        "InstMatmult", "InstNoOp", "InstTensorCopy", "InstActivation",
        "InstMemset", "InstTensorTensor", "InstTensorScalarPtr",
    )
    f = nc.m.functions[0]
    moved = []
    for blk in f.blocks:
        insts = list(blk.instructions)
        donors = {}                      # engine -> wait-free donor stack
        for i in insts:
            eng = i.engine.name
            si = getattr(i, "sync_info", None)
            ws = list(si.on_wait) if si is not None else []
            if len(ws) <= 1:
                if (not ws) and type(i).__name__ in DONOR_TYPES:
                    donors.setdefault(eng, []).append(i)
                continue
            dl = donors.setdefault(eng, [])

            def is_dma(w):
                return (w.ant_name or "").startswith(("DMAHW", "DMASW"))

            def is_self(w):
                return (w.ant_name or "").startswith(eng + "_")

            # 1. PE/DVE run their pipes strictly in order, so a self-wait on
            #    an instruction of the same engine is redundant: drop it.
            if eng in ("PE", "DVE"):
                dropped = [w for w in ws if is_self(w)]
                ws = [w for w in ws if not is_self(w)]
                for w in dropped:
                    moved.append((i.name, "<dropped>", w.ant_name,
                                  w.wait_value, "self"))
            # 2. DMA-lane waits are position-safe (input DMAs fire
            #    unconditionally): park them on any preceding donor.
            engws = [w for w in ws if not is_dma(w)]
            dmaws = [w for w in ws if is_dma(w)]
            while len(engws) + len(dmaws) > 1 and dmaws and dl:
                d = dl.pop()
                dsi = d.sync_info
                dupd = list(dsi.on_update) if dsi is not None else []
                w = dmaws.pop()
                d.sync_info = bass_rust.SyncInfo(on_wait=[w], on_update=dupd)
                moved.append((i.name, d.name, w.ant_name, w.wait_value, "dma"))
            keep = engws + dmaws
            assert len(keep) <= 1, (
                f"_repair_multi_waits: {i.name} ({eng}) still needs "
                f"{[str(w) for w in keep]}; engine-wait moves are unsafe "
                f"(they can cycle on in-order sequencers) -- restructure")
            upd = list(si.on_update)
            i.sync_info = bass_rust.SyncInfo(on_wait=keep, on_update=upd)
    return moved


def _build_module():
    import concourse.bass as bass
    import concourse.tile as tile
    from concourse import mybir

    _patch_tile_drain()
    if SKIP_INIT_BARRIER:
        _patch_skip_init_barrier()

    f32 = mybir.dt.float32
    bf16 = mybir.dt.bfloat16
    nc = bass.Bass(num_swdge_queues=2)

    # ---- DRAM parameters (per core) -------------------------------------
    # pair 0's kq window arrives in two pieces so the first score matmuls
    # start before the whole tensor lands; pairs 1-3 rows 0..63 are per-head
    # unique, rows 64..97 of the key part are replicated on-chip.
    kq0a_d = nc.declare_dram_parameter("kq0a", [AUG, P + 8 * 128], bf16, isOutput=False)
    kq0b_d = nc.declare_dram_parameter("kq0b", [AUG, 9 * 128], bf16, isOutput=False)
    KROWS = 64 if DEDUP else AUG
    kq1_d = nc.declare_dram_parameter("kq1", [KROWS, WKQ], bf16, isOutput=False)
    if DEDUP:
        kq23_d = nc.declare_dram_parameter("kq23", [KROWS, 2 * WKQ], bf16, isOutput=False)
    else:
        kq2_d = nc.declare_dram_parameter("kq2", [KROWS, WKQ], bf16, isOutput=False)
        kq3_d = nc.declare_dram_parameter("kq3", [KROWS, WKQ], bf16, isOutput=False)
    qaugr_d = (nc.declare_dram_parameter("qaugr", [34, 3, P], bf16, isOutput=False)
               if DEDUP else None)
    va0_d = nc.declare_dram_parameter("va0", [128, WVA], bf16, isOutput=False)
    va1_d = nc.declare_dram_parameter("va1", [128, WVA], bf16, isOutput=False)
    va23_d = nc.declare_dram_parameter("va23", [128, 2 * WVA], bf16, isOutput=False)
    out_d = nc.declare_dram_parameter("outt", [PAIRS, P, 65], f32, isOutput=True)

    GROUPS = [(0, 8), (8, 8), (16, 1)]
    SPLIT0 = P + 8 * 128        # pair-0 kq piece boundary

    # expected semaphore fire order for the drain walk: HWDGE lanes 0..7 are
    # the input DMAs in issue order (all fire mid-kernel); the SWDGE lanes
    # carry the out-DMAs, of which the last two fire after the compute tail.
    nc._drain_wait_prio = {
        "DMAHW4": 4, "DMAHW5": 5, "DMAHW6": 6, "DMAHW7": 7,
        "Pool": 20, "Activation": 21, "PE": 22, "DVE": 23,
        "DMAHW0": 30, "DMAHW1": 31, "DMAHW2": 32, "DMAHW3": 33,
    }

    with tile.TileContext(nc) as tc:
        with (
            tc.tile_pool(name="kq", bufs=1) as kqpool,
            tc.tile_pool(name="va", bufs=1) as vapool,
            tc.tile_pool(name="pt", bufs=1) as ptpool,
            tc.tile_pool(name="os", bufs=1) as ospool,
            tc.tile_pool(name="epool", bufs=4) as epool,
            tc.tile_pool(name="sp8sum", bufs=2, space="PSUM") as sp8sum,
            tc.tile_pool(name="sp1sum", bufs=1, space="PSUM") as sp1sum,
            tc.tile_pool(name="apsum", bufs=2, space="PSUM") as apsum,
            tc.tile_pool(name="dpsum", bufs=1, space="PSUM") as dpsum,
        ):
            # ---- static SBUF tensors ------------------------------------
            KQ = kqpool.tile([AUG, PAIRS * WKQ], bf16, name="KQ")
            VA = vapool.tile([128, PAIRS * WVA], bf16, name="VA")
            PT = ptpool.tile([128, PAIRS * WPT], bf16, name="PT")
            OS = ospool.tile([128, PAIRS * 65], f32, name="OS")

            # ---- dep-free warmups ---------------------------------------
            # dummy-claim matmuls each write their OWN psum byte: a shared
            # target would WAW-chain the claims, letting one late-firing
            # claim displace later ones behind their real consumers in
            # Tile's readiness-ordered schedule.
            dumm_all = dpsum.tile([1, 64], f32)
            dumm_ctr = [0]

            def dumm_slot():
                i = dumm_ctr[0]
                dumm_ctr[0] += 1
                return dumm_all[0:1, i:i + 1]

            dumm = dumm_slot()
            # dummy operand: the framework's const pool is initialized in the
            # preamble (before the entry barrier), so reads are dep-free AND
            # defined -- CoreSim rejects reads of uninitialized SBUF
            CB1 = nc.const_aps.aps[(bf16, 1.0)][0:1, 0:1]
            dwarm0 = epool.tile([1, 1], bf16, tag="dwarm")
            nc.tensor.matmul(dumm, lhsT=CB1, rhs=CB1,
                             start=True, stop=True, skip_group_check=True)
            nc.scalar.copy(dwarm0, CB1)
            # DVE warmup doubles as the exp-bias zero column
            ZB = epool.tile([128, 1], bf16, tag="zb")
            nc.vector.memset(ZB, 0.0)

            # ---- input DMAs (SP -> HWDGE), urgency order ----------------
            # exactly 8: one per HWDGE semaphore lane (a 9th would carry a
            # lane-FIFO wait on top of its data wait -> illegal on walrus)
            nc.sync.dma_start(out=KQ[:, :SPLIT0], in_=kq0a_d[:])
            nc.sync.dma_start(out=KQ[:, SPLIT0:WKQ], in_=kq0b_d[:])
            nc.sync.dma_start(out=KQ[:KROWS, WKQ:2 * WKQ], in_=kq1_d[:])
            if DEDUP:
                qaug_dst = KQ[64:, WKQ:].rearrange(
                    "p (w c) -> p w c", c=WKQ)[:, :, :P]
                nc.sync.dma_start(out=qaug_dst, in_=qaugr_d[:])
                nc.sync.dma_start(out=VA[:, :WVA], in_=va0_d[:])
                nc.sync.dma_start(out=KQ[:KROWS, 2 * WKQ:], in_=kq23_d[:])
                nc.sync.dma_start(out=VA[:, WVA:2 * WVA], in_=va1_d[:])
                nc.sync.dma_start(out=VA[:, 2 * WVA:], in_=va23_d[:])
            else:
                dmas = {
                    "va0": lambda: nc.sync.dma_start(out=VA[:, :WVA], in_=va0_d[:]),
                    "kq2": lambda: nc.sync.dma_start(out=KQ[:KROWS, 2 * WKQ:3 * WKQ], in_=kq2_d[:]),
                    "kq3": lambda: nc.sync.dma_start(out=KQ[:KROWS, 3 * WKQ:], in_=kq3_d[:]),
                    "va1": lambda: nc.sync.dma_start(out=VA[:, WVA:2 * WVA], in_=va1_d[:]),
                    "va23": lambda: nc.sync.dma_start(out=VA[:, 2 * WVA:], in_=va23_d[:]),
                }
                order = os.environ.get("K_DMA_ORDER", "va0,kq2,va1,kq3,va23")
                for name in order.split(","):
                    dmas[name]()

            # ---- ACT claims: zero bias column + exp-table preload -------
            dume = epool.tile([1, 1], f32, tag="dume")
            nc.scalar.copy(dume, ZB[0:1, :])
            dume2 = epool.tile([1, 1], f32, tag="dume2")
            nc.scalar.activation(dume2, ZB[0:1, :],
                                 mybir.ActivationFunctionType.Exp,
                                 bias=ZB[0:1, :])

            # ---- DVE: claim pair-0 kq pieces, then replicate aug rows ---
            dumv = epool.tile([1, 2], bf16, tag="dumv")
            if DEDUP:
                nc.vector.tensor_copy(dumv[:, 0:1], KQ[64:65, P:P + 1])            # kq0a
                nc.vector.tensor_copy(dumv[:, 1:2], KQ[64:65, SPLIT0:SPLIT0 + 1])  # kq0b
                nc.vector.nop(nofuse=True)
                for j in range(1, PAIRS):
                    nc.vector.nop(nofuse=True)
                    nc.vector.tensor_copy(
                        KQ[64:, j * WKQ + P:(j + 1) * WKQ],
                        KQ[64:, P:WKQ])

            GL = [GROUPS] * PAIRS
            state = {}

            def claim(engine_matmul_src):
                nc.tensor.matmul(dumm_slot(), lhsT=engine_matmul_src,
                                 rhs=engine_matmul_src,
                                 start=True, stop=True, skip_group_check=True)

            def scores_group(j, gi):
                st = state.setdefault(j, {})
                c0, ng = GL[j][gi]
                w = j * WKQ
                qhat = KQ[:, w:w + P]
                if j == 0:
                    # pair 0 lands in two pieces; claim each as it is used
                    if gi == 0:
                        claim(KQ[0:1, 0:1])
                    elif gi == 1:
                        claim(KQ[0:1, SPLIT0:SPLIT0 + 1])
                elif gi == 0:
                    claim(KQ[0:1, w:w + 1])                # per-head kq DMA
                    if DEDUP:
                        if j == 1:
                            claim(KQ[64:65, w:w + 1])      # qaugr DMA
                        claim(KQ[64:65, w + P:w + P + 1])  # DVE replica j
                # dep-free donor nops: _repair_multi_waits parks displaced
                # waits here, adjacent to the instructions that need them
                nc.tensor.nop(nofuse=True)
                nc.tensor.nop(nofuse=True)
                sp = (sp8sum.tile([128, 1024], f32, name="sp8") if ng > 1
                      else sp1sum.tile([128, 128], f32, name="sp1"))
                for i in range(ng):
                    c = c0 + i
                    nc.tensor.matmul(
                        sp[:, i * P:(i + 1) * P],
                        lhsT=KQ[:, w + P + c * 128:w + P + (c + 1) * 128],
                        rhs=qhat,
                        start=True, stop=True)
                st.setdefault("sps", []).append(sp)

            def exp_group(j, gi):
                st = state[j]
                c0, ng = GL[j][gi]
                sp = st["sps"][gi]
                nc.scalar.activation(
                    PT[:, j * WPT + c0 * P:j * WPT + (c0 + ng) * P],
                    sp[:, :ng * P],
                    mybir.ActivationFunctionType.Exp, bias=ZB)

            def pv_group(j, gi):
                st = state[j]
                c0, ng = GL[j][gi]
                if gi == 0:
                    # claims: va DMA (lane changes at pairs 0,1,2), acc WAR
                    if j == 0:
                        claim(VA[0:1, 0:1])
                    elif j == 1:
                        claim(VA[0:1, WVA:WVA + 1])
                    elif j == 2:
                        claim(VA[0:1, 2 * WVA:2 * WVA + 1])
                    st["acc"] = apsum.tile([P, 65], f32, name="acc")
                    nc.tensor.nop(nofuse=True)
                    if j >= 2:
                        # absorb the WAR wait on the recycled acc buffer
                        nc.tensor.matmul(st["acc"][0:1, 64:65],
                                         lhsT=CB1, rhs=CB1,
                                         start=True, stop=True,
                                         skip_group_check=True)
                for i in range(ng):
                    c = c0 + i
                    nc.tensor.matmul(
                        st["acc"],
                        lhsT=PT[:, j * WPT + c * P:j * WPT + (c + 1) * P],
                        rhs=VA[:, j * WVA + c * 65:j * WVA + (c + 1) * 65],
                        start=(c == 0), stop=(c == NCHUNK - 1))

            def evac_pair(j):
                st = state[j]
                acc_sb = OS[:, j * 65:(j + 1) * 65]
                nc.vector.tensor_copy(acc_sb, st["acc"])
                nc.tensor.matmul(dumm if j == PAIRS - 1 else dumm_slot(),
                                 lhsT=CB1, rhs=CB1,
                                 start=True, stop=True,
                                 skip_group_check=True)
                # out-DMA on a reused HWDGE lane: the lane-FIFO wait is
                # parked on the donor nop by _repair_multi_waits, leaving the
                # DMA with only its DVE data wait
                nc.sync.nop(nofuse=True)
                nc.sync.dma_start(out=out_d[j], in_=acc_sb)

            # software pipeline: scores of pair j+1 interleave with exp/PV of j
            for gi in range(len(GL[0])):
                scores_group(0, gi)
            for j in range(PAIRS):
                nxt = GL[j + 1] if j + 1 < PAIRS else []
                for gi in range(len(GL[j])):
                    exp_group(j, gi)
                    pv_group(j, gi)
                    if gi < len(nxt):
                        scores_group(j + 1, gi)
                evac_pair(j)

    _repair_multi_waits(nc)
    return nc


def _get_nc():
    if "nc" not in _COMPILED:
        _COMPILED["nc"] = _build_module()
    return _COMPILED["nc"]


def kernel(pool_q, pool_k, pool_v, x_q, x_k, x_v, bias_slopes, regions,
           t_mask, n_mask, max_n):
    from concourse.bass_utils import run_bass_kernel_spmd

    kqa, va = _host_prep(
        np.asarray(pool_q, np.float32), np.asarray(pool_k, np.float32),
        np.asarray(pool_v, np.float32), np.asarray(x_q, np.float32),
        np.asarray(x_k, np.float32), np.asarray(x_v, np.float32),
        np.asarray(bias_slopes, np.float32), np.asarray(regions))

    SPLIT0 = P + 8 * 128
    in_maps = []
    for c in range(NCORES):
        b, h0 = c // 4, 4 * (c % 4)
        kq = kqa[b]                                     # [H, 98, WKQ]
        kr = 64 if DEDUP else AUG
        m = {
            "kq0a": np.ascontiguousarray(kq[h0, :, :SPLIT0]),
            "kq0b": np.ascontiguousarray(kq[h0, :, SPLIT0:]),
            "kq1": np.ascontiguousarray(kq[h0 + 1, :kr]),
        }
        if DEDUP:
            m["kq23"] = np.ascontiguousarray(
                np.swapaxes(kq[h0 + 2:h0 + 4, :kr], 0, 1).reshape(kr, 2 * WKQ))
        else:
            m["kq2"] = np.ascontiguousarray(kq[h0 + 2, :kr])
            m["kq3"] = np.ascontiguousarray(kq[h0 + 3, :kr])
        m.update({
            "va0": np.ascontiguousarray(va[b, h0]),
            "va1": np.ascontiguousarray(va[b, h0 + 1]),
            "va23": np.ascontiguousarray(
                np.swapaxes(va[b, h0 + 2:h0 + 4], 0, 1).reshape(128, 2 * WVA)),
        })
        if DEDUP:
            m["qaugr"] = np.ascontiguousarray(
                np.swapaxes(kq[h0 + 1:h0 + 4, 64:, :P], 0, 1))
        in_maps.append(m)

    nc = _get_nc()
    res = run_bass_kernel_spmd(
        nc, in_maps, core_ids=list(range(NCORES)),
        trace=bool(int(os.environ.get("KERNEL_TRACE", "0"))))
    _COMPILED["last_result"] = res

    out = np.empty((B, H, P, 64), np.float32)
    for c in range(NCORES):
        b, h0 = c // 4, 4 * (c % 4)
        ot = res.results[c]["outt"]                        # [PAIRS, P, 65]
        out[b, h0:h0 + PAIRS] = ot[:, :, :64] / ot[:, :, 64:65]
    return out
